# revision 1
# baseline (speedup 1.0000x reference)
"""Trainium2 Bass kernel for nn_CombinedGraphLayer (LSH-binned GHConv message passing).

Contract: kernel(**inputs) takes FULL inputs (x [16,12800,256], msk [16,12800],
training scalar + weights), returns FULL output [16,12800,256].

Strategy: pure data-parallel over batch (2 batches per NeuronCore x 8 cores).
Per batch, on device:
  phase A  (per 128-point chunk): layernorm -> ffn_dist -> LSH argmax key;
           accumulate per-chunk key histograms; keep z*m / x_dist resident.
  phase A2: counting-sort ranks via prefix sums (matmul + scan tricks);
           indirect-scatter packed rows into sorted order (DRAM).
  phase B  (per 128-point bin): pairwise gaussian adjacency + 2 GHConv
           layers; indirect-scatter results back to original row order.

gamma/beta of the layernorm are folded into the ffn/GHConv weights on host
(pure input marshaling); msk is converted to float once on host.
"""

import numpy as np

import concourse.bass as bass
import concourse.tile as tile
from concourse import mybir
from concourse.bass_utils import run_bass_kernel_spmd
from concourse.masks import make_identity

dt = mybir.dt
OP = mybir.AluOpType
AF = mybir.ActivationFunctionType
IOA = bass.IndirectOffsetOnAxis

ABLATION = ""  # bench knob: "noB", "noscat", "nosort" (timing experiments only)

F = 256       # feature dim
D = 128       # distance dim
BIN = 128

# packed row layout (fp32): [ zm(0:256) | xd(256:384) | m(384) | idx(385) | pad ]
RW = 388
COL_M = 384
COL_IDX = 385


def split_excess_waits(nc):
    """This walrus build rejects instructions carrying more than a couple of
    sem waits (1 for CTRL-class like Drain, ~2 for compute). Move excess
    waits onto extra Drains inserted just before, on the same engine."""
    for f in nc.m.functions:
        for b in f.blocks:
            new_insts = []
            for inst in b.instructions:
                si = getattr(inst, "sync_info", None)
                ow = list(si.on_wait) if si is not None and si.on_wait else []
                limit = 1
                if len(ow) > limit and inst.engine is not None:
                    keep = ow[-limit:]
                    for w in ow[:-limit]:
                        d = mybir.InstNoOp(
                            name=nc.get_next_instruction_name(), ins=[], outs=[]
                        )
                        d.engine = inst.engine
                        d.sync_info = mybir.SyncInfo(on_wait=[w], on_update=[])
                        new_insts.append(d)
                    si.on_wait = keep
                new_insts.append(inst)
            b.instructions = new_insts


def build(nb, nch, ghconv_dtype=dt.float32):
    """Build the Bass module for nb batches of nch 128-point chunks each."""
    NP = nch * BIN            # points per batch
    NBINS = nch               # bins == chunks
    CB = NBINS // 2           # used codebook columns
    NK = 2 * NBINS - 1        # distinct sort keys: 0 .. NK-1
    NKP = NK + 1              # padded width
    f32 = dt.float32
    bf16 = dt.bfloat16
    use_r = ghconv_dtype == dt.float32r
    gdt = ghconv_dtype

    nc = bass.Bass("TRN2", target_bir_lowering=False, debug=False)

    x_in = nc.dram_tensor("x", [nb * NP, F], f32, kind="ExternalInput").ap()
    m_in = nc.dram_tensor("m", [nb * NP, 1], f32, kind="ExternalInput").ap()
    wspec = [
        ("W1g", [F, D]), ("b1gb", [1, D]), ("W2", [D, D]), ("b2", [1, D]),
        ("CB", [D, CB]),
        ("th0", [F, F]), ("Wh0", [F, F]), ("Wt0", [F, F]),
        ("bth0", [1, F]), ("bhh0", [1, F]), ("bgt0", [1, F]),
        ("th1", [F, F]), ("Wh1", [F, F]), ("Wt1", [F, F]), ("bt1", [1, F]),
    ]
    wdram = {n: nc.dram_tensor(n, s, f32, kind="ExternalInput").ap() for n, s in wspec}
    outs = [nc.dram_tensor(f"out{b}", [NP, F], f32, kind="ExternalOutput").ap()
            for b in range(nb)]
    psort = [nc.dram_tensor(f"psort{b}", [NP, RW], f32, kind="Internal").ap()
             for b in range(nb)]

    with tile.TileContext(nc) as tc:
        with tc.tile_pool(name="init", bufs=1) as ip:
            ident = ip.tile([128, 128], f32)
            make_identity(nc, ident[:])
            eps_t = ip.tile([128, 1], f32)
            nc.vector.memset(eps_t[:], 1e-6)
            iota_p_i = ip.tile([128, 1], dt.int32)
            nc.gpsimd.iota(iota_p_i[:], [[0, 1]], base=0, channel_multiplier=1)
            iota_p_f = ip.tile([128, 1], f32)
            nc.vector.tensor_copy(iota_p_f[:], iota_p_i[:])
            iota_row_i = ip.tile([128, NKP], dt.int32)
            nc.gpsimd.iota(iota_row_i[:], [[1, NKP]], base=0, channel_multiplier=0)
            iota_row_f = ip.tile([128, NKP], f32)
            nc.vector.tensor_copy(iota_row_f[:], iota_row_i[:])
            # strictly-lower mask Tp[p', p] = 1 if p' < p  (for within-chunk cumsum)
            iota_r128_i = ip.tile([128, 128], dt.int32)
            nc.gpsimd.iota(iota_r128_i[:], [[1, 128]], base=0, channel_multiplier=0)
            iota_r128_f = ip.tile([128, 128], f32)
            nc.vector.tensor_copy(iota_r128_f[:], iota_r128_i[:])
            Tp_bf = ip.tile([128, 128], bf16)
            nc.vector.tensor_scalar(
                out=Tp_bf[:], in0=iota_r128_f[:], scalar1=iota_p_f[:],
                scalar2=None, op0=OP.is_gt)
            ones_col_bf = ip.tile([128, 1], bf16)
            nc.vector.memset(ones_col_bf[:], 1.0)
            ones_row_f = ip.tile([1, 128], f32)
            nc.vector.memset(ones_row_f[:], 1.0)
            ones_row_g = ip.tile([1, 128], gdt)
            if gdt == f32:
                nc.vector.memset(ones_row_g[:], 1.0)
            else:
                nc.vector.tensor_copy(ones_row_g[:], ones_row_f[:])

            # weights to SBUF
            wsb = {}
            for n, s in wspec:
                wdt = f32
                if n in ("th0", "Wh0", "Wt0", "th1", "Wh1", "Wt1",
                         "bth0", "bhh0", "bgt0", "bt1"):
                    wdt = gdt
                shp = ([128, s[0] // 128, s[1]] if s[0] > 128 else list(s))
                src = (wdram[n].rearrange("(c p) m -> p c m", p=128)
                       if s[0] > 128 else wdram[n][:])
                if wdt == f32:
                    t = ip.tile(shp, f32, tag=f"w_{n}")
                    nc.gpsimd.dma_start(out=t[:], in_=src)
                else:
                    stg = ip.tile(shp, f32, tag="w_stage")
                    nc.gpsimd.dma_start(out=stg[:], in_=src)
                    t = ip.tile(shp, wdt, tag=f"w_{n}")
                    nc.vector.tensor_copy(t[:], stg[:])
                wsb[n] = t

            for b in range(nb):
                _one_batch(tc, nc, b, nb, nch, NP, NBINS, CB, NK, NKP,
                           x_in, m_in, wsb, outs[b], psort[b],
                           ident, eps_t, iota_p_f, iota_row_f, Tp_bf,
                           ones_col_bf, ones_row_f, ones_row_g, gdt, use_r)

    split_excess_waits(nc)
    return nc


def _one_batch(tc, nc, b, nb, nch, NP, NBINS, CB, NK, NKP,
               x_in, m_in, wsb, out_d, psort_d,
               ident, eps_t, iota_p_f, iota_row_f, Tp_bf,
               ones_col_bf, ones_row_f, ones_row_g, gdt, use_r):
    f32 = dt.float32
    bf16 = dt.bfloat16
    KHI = float(NBINS - 1)
    if use_r:
        def R(ap):
            return ap.bitcast(dt.float32r)
    else:
        def R(ap):
            return ap

    with tc.tile_pool(name=f"res{b}", bufs=1) as rp, \
         tc.tile_pool(name=f"resps{b}", bufs=1, space="PSUM") as rpp:
        packed = rp.tile([128, nch, RW], f32)     # resident z*m / xd / m / idx
        key_all = rp.tile([128, nch], f32)
        rank_f = rp.tile([128, nch], f32)
        rank_u = rp.tile([128, nch], dt.uint32)
        T_lo = rpp.tile([NBINS, nch], f32, space="PSUM")
        T_hi = rpp.tile([NBINS - 1, nch], f32, space="PSUM")

        # ---------------- phase A ----------------
        with tc.tile_pool(name=f"pa{b}", bufs=3) as pa, \
             tc.tile_pool(name=f"paps{b}", bufs=1, space="PSUM") as pap:
            for c in range(nch):
                row0 = b * NP + c * 128
                x_t = pa.tile([128, F], f32)
                nc.sync.dma_start(out=x_t[:], in_=x_in[row0:row0 + 128, :])
                nc.sync.dma_start(out=packed[:, c, COL_M:COL_M + 1],
                                  in_=m_in[row0:row0 + 128, :])
                m_ap = packed[:, c, COL_M:COL_M + 1]

                st = pa.tile([128, 6], f32)
                nc.vector.bn_stats(out=st[:], in_=x_t[:])
                mv = pa.tile([128, 2], f32)
                nc.vector.bn_aggr(out=mv[:], in_=st[:])
                nc.scalar.activation(out=mv[:, 1:2], in_=mv[:, 1:2],
                                     func=AF.Sqrt, bias=eps_t[:])
                nc.vector.reciprocal(out=mv[:, 1:2], in_=mv[:, 1:2])
                z_t = pa.tile([128, F], f32)
                nc.vector.tensor_scalar(
                    out=z_t[:], in0=x_t[:], scalar1=mv[:, 0:1],
                    scalar2=mv[:, 1:2], op0=OP.subtract, op1=OP.mult)
                # zm into packed (gpsimd: SBUF only)
                nc.gpsimd.tensor_scalar_mul(packed[:, c, 0:F], z_t[:], m_ap)

                # zT (feature-major) for the ffn matmuls
                zT_ps = pap.tile([128, 2, 128], f32, space="PSUM")
                for k in range(2):
                    nc.tensor.transpose(zT_ps[:, k, :],
                                        z_t[:, k * 128:(k + 1) * 128], ident[:])
                zT_sb = pa.tile([128, 2, 128], f32)
                nc.scalar.activation(out=zT_sb[:], in_=zT_ps[:], func=AF.Copy)

                # hT = W1g^T zT + b1gb  (feature-major [D, pts])
                h_ps = pap.tile([128, 128], f32, space="PSUM")
                nc.tensor.matmul(h_ps[:], lhsT=wsb["W1g"][:, 0, :],
                                 rhs=zT_sb[:, 0, :], start=True, stop=False)
                nc.tensor.matmul(h_ps[:], lhsT=wsb["W1g"][:, 1, :],
                                 rhs=zT_sb[:, 1, :], start=False, stop=False)
                nc.tensor.matmul(h_ps[:], lhsT=wsb["b1gb"][:],
                                 rhs=ones_row_f[:], start=False, stop=True)
                # elu
                e_t = pa.tile([128, 128], f32)
                nc.vector.tensor_scalar_min(e_t[:], h_ps[:], 0.0)
                nc.scalar.activation(out=e_t[:], in_=e_t[:], func=AF.Exp)
                r_t = pa.tile([128, 128], f32)
                nc.scalar.activation(out=r_t[:], in_=h_ps[:], func=AF.Relu)
                hTe = pa.tile([128, 128], f32)
                nc.vector.scalar_tensor_tensor(
                    out=hTe[:], in0=e_t[:], scalar=-1.0, in1=r_t[:],
                    op0=OP.add, op1=OP.add)

                # xdT = W2^T hTe + b2
                xdT_ps = pap.tile([128, 128], f32, space="PSUM")
                nc.tensor.matmul(xdT_ps[:], lhsT=wsb["W2"][:], rhs=hTe[:],
                                 start=True, stop=False)
                nc.tensor.matmul(xdT_ps[:], lhsT=wsb["b2"][:],
                                 rhs=ones_row_f[:], start=False, stop=True)
                xdT_sb = pa.tile([128, 128], f32)
                nc.scalar.activation(out=xdT_sb[:], in_=xdT_ps[:], func=AF.Copy)
                # xd point-major into packed
                xd_ps = pap.tile([128, 128], f32, space="PSUM")
                nc.tensor.transpose(xd_ps[:], xdT_sb[:], ident[:])
                nc.vector.tensor_copy(packed[:, c, F:F + 128], xd_ps[:])
                # idx column
                nc.vector.tensor_scalar_add(
                    packed[:, c, COL_IDX:COL_IDX + 1], iota_p_f[:], float(c * 128))

                # mul = xd @ codebook  (point-major [pts, CB])
                mul_ps = pap.tile([128, CB], f32, space="PSUM")
                nc.tensor.matmul(mul_ps[:], lhsT=xdT_sb[:], rhs=wsb["CB"][:],
                                 start=True, stop=True)
                cmul = pa.tile([128, 2 * CB], f32)
                nc.scalar.activation(out=cmul[:, 0:CB], in_=mul_ps[:], func=AF.Copy)
                nc.scalar.activation(out=cmul[:, CB:2 * CB], in_=mul_ps[:],
                                     func=AF.Copy, scale=-1.0)
                mx8 = pa.tile([128, 8], f32)
                nc.vector.max(out=mx8[:], in_=cmul[:])
                ix8 = pa.tile([128, 8], dt.uint32)
                nc.vector.max_index(out=ix8[:], in_max=mx8[:], in_values=cmul[:])
                idxf = pa.tile([128, 1], f32)
                nc.vector.tensor_copy(idxf[:], ix8[:, 0:1])
                # key = argmax + (NBINS-1)*(1-m)
                nc.vector.scalar_tensor_tensor(
                    out=key_all[:, c:c + 1], in0=m_ap, scalar=-KHI, in1=idxf[:],
                    op0=OP.mult, op1=OP.add)
                nc.vector.tensor_scalar_add(key_all[:, c:c + 1],
                                            key_all[:, c:c + 1], KHI)
                # onehot -> per-chunk histogram columns
                oh = pa.tile([128, NKP], bf16)
                nc.vector.tensor_scalar(
                    out=oh[:], in0=iota_row_f[:, 0:NKP], scalar1=key_all[:, c:c + 1],
                    scalar2=None, op0=OP.is_equal)
                nc.tensor.matmul(T_lo[:, c:c + 1], lhsT=oh[:, 0:NBINS],
                                 rhs=ones_col_bf[:], start=True, stop=True)
                nc.tensor.matmul(T_hi[:, c:c + 1], lhsT=oh[:, NBINS:NK],
                                 rhs=ones_col_bf[:], start=True, stop=True)

        # ---------------- phase A2: ranks + sort scatter ----------------
        with tc.tile_pool(name=f"pa2{b}", bufs=2) as p2, \
             tc.tile_pool(name=f"pa2ps{b}", bufs=1, space="PSUM") as p2p:
            Tl_sb = p2.tile([NBINS, nch], f32)
            nc.scalar.activation(out=Tl_sb[:], in_=T_lo[:], func=AF.Copy)
            Th_sb = p2.tile([NBINS - 1, nch], f32)
            nc.scalar.activation(out=Th_sb[:], in_=T_hi[:], func=AF.Copy)
            # inclusive scan along chunks
            Sl_in = p2.tile([NBINS, nch], f32)
            nc.vector.tensor_tensor_scan(Sl_in[:], Tl_sb[:], Tl_sb[:], 0.0,
                                         OP.add, OP.bypass)
            Sh_in = p2.tile([NBINS - 1, nch], f32)
            nc.vector.tensor_tensor_scan(Sh_in[:], Th_sb[:], Th_sb[:], 0.0,
                                         OP.add, OP.bypass)
            # exclusive
            Sl_ex = p2.tile([NBINS, nch], f32)
            nc.vector.tensor_sub(Sl_ex[:], Sl_in[:], Tl_sb[:])
            Sh_ex = p2.tile([NBINS - 1, nch], f32)
            nc.vector.tensor_sub(Sh_ex[:], Sh_in[:], Th_sb[:])
            # grand totals -> key-offsets (exclusive cumsum over keys)
            grow_ps = p2p.tile([1, NKP], f32, space="PSUM")
            nc.tensor.transpose(grow_ps[:, 0:NBINS], Sl_in[:, nch - 1:nch],
                                ident[0:NBINS, 0:NBINS])
            nc.tensor.transpose(grow_ps[:, NBINS:NK], Sh_in[:, nch - 1:nch],
                                ident[0:NBINS - 1, 0:NBINS - 1])
            grow_sb = p2.tile([1, NKP], f32)
            nc.vector.memset(grow_sb[:], 0.0)
            nc.scalar.activation(out=grow_sb[:, 0:NK], in_=grow_ps[:, 0:NK],
                                 func=AF.Copy)
            ginc = p2.tile([1, NKP], f32)
            nc.vector.tensor_tensor_scan(ginc[:], grow_sb[:], grow_sb[:], 0.0,
                                         OP.add, OP.bypass)
            gexc = p2.tile([1, NKP], f32)
            nc.vector.tensor_sub(gexc[:], ginc[:], grow_sb[:])
            offs_ps = p2p.tile([NBINS, 2], f32, space="PSUM")
            nc.tensor.transpose(offs_ps[:, 0:1], gexc[:, 0:NBINS], ident[0:1, 0:1])
            nc.tensor.transpose(offs_ps[0:NBINS - 1, 1:2], gexc[:, NBINS:NK],
                                ident[0:1, 0:1])
            offs_sb = p2.tile([NBINS, 2], f32)
            nc.scalar.activation(out=offs_sb[:], in_=offs_ps[:], func=AF.Copy)
            nc.vector.tensor_scalar_add(Sl_ex[:], Sl_ex[:], offs_sb[:, 0:1])
            nc.vector.tensor_scalar_add(Sh_ex[:], Sh_ex[:],
                                        offs_sb[0:NBINS - 1, 1:2])
            # St[c, k] = base for chunk c / key k
            St_ps = p2p.tile([nch, NKP], f32, space="PSUM")
            nc.tensor.transpose(St_ps[:, 0:NBINS], Sl_ex[:],
                                ident[0:NBINS, 0:NBINS])
            nc.tensor.transpose(St_ps[:, NBINS:NK], Sh_ex[:],
                                ident[0:NBINS - 1, 0:NBINS - 1])
            St_sb = p2.tile([nch, NKP], f32)
            nc.vector.memset(St_sb[:], 0.0)
            nc.scalar.activation(out=St_sb[:, 0:NK], in_=St_ps[:, 0:NK],
                                 func=AF.Copy)

            for c in range(nch):
                oh2 = p2.tile([128, NKP], bf16)
                nc.vector.tensor_scalar(
                    out=oh2[:], in0=iota_row_f[:, 0:NKP],
                    scalar1=key_all[:, c:c + 1], scalar2=None, op0=OP.is_equal)
                St_row = p2.tile([1, NKP], f32)
                nc.sync.dma_start(out=St_row[:], in_=St_sb[c:c + 1, :])
                C_ps = p2p.tile([128, NKP], f32, space="PSUM")
                nc.tensor.matmul(C_ps[:], lhsT=Tp_bf[:], rhs=oh2[:],
                                 start=True, stop=False)
                nc.tensor.matmul(C_ps[:], lhsT=ones_row_f[:],
                                 rhs=St_row[:], start=False, stop=True)
                scr = p2.tile([128, NKP], f32)
                nc.vector.tensor_tensor(out=scr[:], in0=oh2[:], in1=C_ps[:],
                                        op=OP.mult)
                nc.vector.tensor_reduce(
                    out=rank_f[:, c:c + 1], in_=scr[:],
                    axis=mybir.AxisListType.X, op=OP.add)
            nc.vector.tensor_copy(rank_u[:], rank_f[:])
            for c in range(nch):
                if "nosort" in ABLATION:
                    nc.sync.dma_start(
                        out=psort_d[c * 128:(c + 1) * 128, :],
                        in_=packed[:, c, :])
                else:
                    nc.gpsimd.indirect_dma_start(
                        out=psort_d[:],
                        out_offset=IOA(ap=rank_u[:, c:c + 1], axis=0),
                        in_=packed[:, c, :], in_offset=None)

    # ---------------- phase B: adjacency + GHConv per bin ----------------
    if "noB" in ABLATION:
        return
    with tc.tile_pool(name=f"pb{b}", bufs=4) as pb, \
         tc.tile_pool(name=f"pbps{b}", bufs=1, space="PSUM") as pbp:
        for s in range(NBINS):
            pk = pb.tile([128, RW], f32)
            nc.sync.dma_start(out=pk[:], in_=psort_d[s * 128:(s + 1) * 128, :])
            m_ap = pk[:, COL_M:COL_M + 1]
            # V cols: [na, one, one, na, m]; transposed pair/row tiles all
            # land at partition base 0 (matmul requires equal bases).
            V = pb.tile([128, 5], f32)
            sq = pb.tile([128, 128], f32)
            nc.scalar.activation(out=sq[:], in_=pk[:, F:F + 128],
                                 func=AF.Square, accum_out=V[:, 0:1])
            nc.gpsimd.memset(V[:, 1:3], 1.0)
            nc.gpsimd.tensor_copy(V[:, 3:4], V[:, 0:1])
            nc.gpsimd.tensor_copy(V[:, 4:5], m_ap)
            vt_ps = pbp.tile([2, 384], f32, space="PSUM")
            nc.tensor.transpose(vt_ps[0:2, 0:128], V[:, 0:2], ident[:])
            VTa = pb.tile([2, 128], f32)
            nc.scalar.activation(out=VTa[:], in_=vt_ps[0:2, 0:128],
                                 func=AF.Copy)
            nc.tensor.transpose(vt_ps[0:2, 128:256], V[:, 2:4], ident[:])
            VTb = pb.tile([2, 128], f32)
            nc.scalar.activation(out=VTb[:], in_=vt_ps[0:2, 128:256],
                                 func=AF.Copy)
            nc.tensor.transpose(vt_ps[0:1, 256:384], V[:, 4:5], ident[:])
            mT_sb = pb.tile([1, 128], f32)
            nc.scalar.activation(out=mT_sb[:], in_=vt_ps[0:1, 256:384],
                                 func=AF.Copy)
            # d2 = na_i - 2 xd xd^T + na_j ; M2 = m_i m_j
            adj_ps = pbp.tile([128, 384], f32, space="PSUM")
            xdT_ps = adj_ps[:, 0:128]
            d2_ps = adj_ps[:, 128:256]
            M2_ps = adj_ps[:, 256:384]
            nc.tensor.transpose(xdT_ps, pk[:, F:F + 128], ident[:])
            xdT = pb.tile([128, 128], f32)
            nc.scalar.activation(out=xdT[:], in_=xdT_ps, func=AF.Copy)
            xdTm2 = pb.tile([128, 128], f32)
            nc.scalar.activation(out=xdTm2[:], in_=xdT_ps, func=AF.Copy,
                                 scale=-2.0)
            nc.tensor.matmul(d2_ps, lhsT=xdTm2[:], rhs=xdT[:],
                             start=True, stop=False)
            nc.tensor.matmul(d2_ps, lhsT=VTa[:], rhs=VTb[:],
                             start=False, stop=True)
            nc.tensor.matmul(M2_ps, lhsT=mT_sb[:], rhs=mT_sb[:],
                             start=True, stop=True)
            dsc = pb.tile([128, 128], f32)
            nc.vector.tensor_scalar_max(dsc[:], d2_ps[:], 1e-6)
            nc.scalar.activation(out=dsc[:], in_=dsc[:], func=AF.Sqrt)
            nc.scalar.activation(out=dsc[:], in_=dsc[:], func=AF.Exp,
                                 scale=-0.1)
            dm = pb.tile([128, 128], gdt)
            ind = pb.tile([128, 1], f32)
            nc.vector.scalar_tensor_tensor(
                out=dm[:], in0=dsc[:], scalar=1.0, in1=M2_ps[:],
                op0=OP.mult, op1=OP.mult, accum_out=ind[:])
            nrm = pb.tile([128, 1], f32)
            nc.scalar.activation(out=nrm[:], in_=ind[:], func=AF.Sqrt,
                                 bias=eps_t[:])
            nc.vector.reciprocal(nrm[:], nrm[:])
            nc.vector.tensor_mul(nrm[:], nrm[:], m_ap)

            xb_ap = pk[:, 0:F]
            for li in range(2):
                sfx = "0" if li == 0 else "1"
                mm1 = pbp.tile([128, 512], f32, space="PSUM")
                mm2 = pbp.tile([128, 512], f32, space="PSUM")
                gat_ps = pbp.tile([128, F], f32, space="PSUM")
                xmT_ps = mm1[:, 0:256]
                hom2_ps = mm1[:, 256:512]
                hom_ps = mm2[:, 0:256]
                het_ps = mm2[:, 256:512]
                for k in range(2):
                    nc.tensor.transpose(
                        xmT_ps.rearrange("p (c q) -> p c q", q=128)[:, k, :],
                        xb_ap[:, k * 128:(k + 1) * 128], ident[:])
                xmT = pb.tile([128, 2, 128], gdt)
                nc.scalar.activation(out=xmT[:], in_=xmT_ps, func=AF.Copy)
                mT = mT_sb[:]
                if gdt != f32:
                    mTg = pb.tile([1, 128], gdt)
                    nc.vector.tensor_copy(mTg[:], mT_sb[:])
                    mT = mTg[:]
                # keep each PSUM accumulation group's matmuls consecutive
                for dst, wn, bias in (
                    (hom_ps, "th" + sfx, "bth0" if li == 0 else None),
                    (het_ps, "Wh" + sfx, "bhh0" if li == 0 else None),
                    (gat_ps[:], "Wt" + sfx,
                     "bgt0" if li == 0 else "bt1"),
                ):
                    for k in range(2):
                        nc.tensor.matmul(
                            dst, lhsT=R(xmT[:, k, :]), rhs=R(wsb[wn][:, k, :]),
                            start=(k == 0), stop=(k == 1 and bias is None))
                    if bias is not None:
                        blhs = mT if li == 0 else ones_row_g[:]
                        nc.tensor.matmul(dst, lhsT=R(blhs), rhs=R(wsb[bias][:]),
                                         start=False, stop=True)
                fh1 = pb.tile([128, F], gdt)
                nc.vector.tensor_scalar_mul(fh1[:], hom_ps[:], nrm[:])
                nc.tensor.matmul(hom2_ps[:], lhsT=R(dm[:]), rhs=R(fh1[:]),
                                 start=True, stop=True)
                gate = pb.tile([128, F], f32)
                nc.scalar.activation(out=gate[:], in_=gat_ps[:], func=AF.Sigmoid)
                fh2 = pb.tile([128, F], f32)
                nc.vector.tensor_scalar_mul(fh2[:], hom2_ps[:], nrm[:])
                nc.vector.tensor_sub(fh2[:], fh2[:], het_ps[:])
                nc.vector.tensor_mul(gate[:], gate[:], fh2[:])
                nc.vector.tensor_add(fh2[:], gate[:], het_ps[:])  # pre-act
                emin = pb.tile([128, F], f32)
                nc.gpsimd.tensor_scalar_min(emin[:], fh2[:], 0.0)
                nc.scalar.activation(out=emin[:], in_=emin[:], func=AF.Exp)
                er = pb.tile([128, F], f32)
                nc.scalar.activation(out=er[:], in_=fh2[:], func=AF.Relu)
                nc.vector.scalar_tensor_tensor(
                    out=emin[:], in0=emin[:], scalar=-1.0, in1=er[:],
                    op0=OP.add, op1=OP.add)
                out_t = pb.tile([128, F], f32)
                nc.gpsimd.tensor_scalar_mul(out_t[:], emin[:], m_ap)
                xb_ap = out_t[:]
            if "noscat" in ABLATION:
                nc.sync.dma_start(out=out_d[s * 128:(s + 1) * 128, :], in_=xb_ap)
            else:
                idx_u = pb.tile([128, 1], dt.uint32)
                nc.vector.tensor_copy(idx_u[:], pk[:, COL_IDX:COL_IDX + 1])
                nc.gpsimd.indirect_dma_start(
                    out=out_d[:], out_offset=IOA(ap=idx_u[:, 0:1], axis=0),
                    in_=xb_ap, in_offset=None)


def _fold_weights(inputs):
    g = inputs["ln_gamma"].astype(np.float32)
    be = inputs["ln_beta"].astype(np.float32)
    W1 = inputs["W1"].astype(np.float32)
    b1 = inputs["b1"].astype(np.float32)
    w = {
        "W1g": g[:, None] * W1,
        "b1gb": (b1 + be @ W1)[None, :],
        "W2": inputs["W2"].astype(np.float32),
        "b2": inputs["b2"].astype(np.float32)[None, :],
        "th1": inputs["th1"].astype(np.float32),
        "Wh1": inputs["Wh1"].astype(np.float32),
        "Wt1": inputs["Wt1"].astype(np.float32),
        "bt1": inputs["bt1"].astype(np.float32)[None, :],
    }
    for nm in ("th0", "Wh0", "Wt0"):
        w[nm] = g[:, None] * inputs[nm].astype(np.float32)
    w["bth0"] = (be @ inputs["th0"].astype(np.float32))[None, :]
    w["bhh0"] = (be @ inputs["Wh0"].astype(np.float32))[None, :]
    w["bgt0"] = (inputs["bt0"].astype(np.float32) +
                 be @ inputs["Wt0"].astype(np.float32))[None, :]
    return {k: np.ascontiguousarray(v, dtype=np.float32) for k, v in w.items()}


_BUILD_CACHE = {}


def _get_nc(nb, nch, ghconv_dtype=dt.float32):
    key = (nb, nch, ghconv_dtype, ABLATION)
    if key not in _BUILD_CACHE:
        _BUILD_CACHE[key] = build(nb, nch, ghconv_dtype)
    return _BUILD_CACHE[key]


_RUNNER_CACHE = {}


def _get_runner(nb, nch, ghconv_dtype, n_cores):
    """Cached jitted SPMD executor (re-jitting per call costs seconds)."""
    key = (nb, nch, ghconv_dtype, n_cores)
    if key in _RUNNER_CACHE:
        return _RUNNER_CACHE[key]
    import jax
    from jax.sharding import Mesh, PartitionSpec
    from jax.experimental.shard_map import shard_map
    from concourse import bass2jax

    bass2jax.install_neuronx_cc_hook()
    nc = _get_nc(nb, nch, ghconv_dtype)
    partition_name = (nc.partition_id_tensor.name
                      if nc.partition_id_tensor else None)
    in_names, out_names, out_avals, zero_shapes = [], [], [], []
    for alloc in nc.m.functions[0].allocations:
        if not isinstance(alloc, mybir.MemoryLocationSet):
            continue
        name = alloc.memorylocations[0].name
        if alloc.kind == "ExternalInput":
            if name != partition_name:
                in_names.append(name)
        elif alloc.kind == "ExternalOutput":
            out_names.append(name)
            shape = tuple(alloc.tensor_shape)
            dtype = mybir.dt.np(alloc.dtype)
            out_avals.append(jax.core.ShapedArray(shape, dtype))
            zero_shapes.append((shape, dtype))
    n_params = len(in_names)
    all_names = in_names + out_names
    if partition_name is not None:
        all_names = all_names + [partition_name]
    def _body(*args):
        operands = list(args)
        if partition_name is not None:
            operands.append(bass2jax.partition_id_tensor())
        outs = bass2jax._bass_exec_p.bind(
            *operands,
            out_avals=tuple(out_avals),
            in_names=tuple(all_names),
            out_names=tuple(out_names),
            lowering_input_output_aliases=(),
            sim_require_finite=True,
            sim_require_nnan=True,
            nc=nc,
        )
        return tuple(outs)

    devices = jax.devices()[:n_cores]
    mesh = Mesh(np.asarray(devices), ("core",))
    in_specs = (PartitionSpec("core"),) * (n_params + len(out_names))
    out_specs = (PartitionSpec("core"),) * len(out_names)
    sharded = jax.jit(
        shard_map(_body, mesh=mesh, in_specs=in_specs, out_specs=out_specs,
                  check_rep=False),
        keep_unused=True)
    # zero output buffers staged on device ONCE and reused read-only
    from jax.sharding import NamedSharding
    shard = NamedSharding(mesh, PartitionSpec("core"))
    dev_zeros = [
        jax.device_put(np.zeros((n_cores * s0[0], *s0[1:]), d), shard)
        for s0, d in zero_shapes]
    runner = (sharded, in_names, out_names, out_avals, dev_zeros)
    _RUNNER_CACHE[key] = runner
    return runner


def run_cached(in_maps, nb, nch, ghconv_dtype, n_cores):
    sharded, in_names, out_names, out_avals, dev_zeros = _get_runner(
        nb, nch, ghconv_dtype, n_cores)
    concat_in = [
        np.concatenate([np.asarray(in_maps[c][n]) for c in range(n_cores)], axis=0)
        for n in in_names]
    out_arrs = sharded(*concat_in, *dev_zeros)
    return [
        {n: np.asarray(out_arrs[i]).reshape(n_cores, *out_avals[i].shape)[c]
         for i, n in enumerate(out_names)}
        for c in range(n_cores)]


def run(inputs, nb, nch, n_cores, ghconv_dtype=dt.float32, trace=False):
    """inputs: dict with x [Btot, NP, F] float32, msk [Btot, NP] bool + weights.
    Btot must equal n_cores * nb."""
    NP = nch * BIN
    x = np.ascontiguousarray(inputs["x"], dtype=np.float32)
    mf = np.ascontiguousarray(inputs["msk"], dtype=np.float32)[..., None]
    Btot = x.shape[0]
    assert Btot == n_cores * nb
    w = _fold_weights(inputs)
    w["CB"] = np.ascontiguousarray(
        inputs["codebook"][:, :NP // BIN // 2], dtype=np.float32)

    in_maps = []
    for core in range(n_cores):
        im = dict(w)
        im["x"] = x[core * nb:(core + 1) * nb].reshape(nb * NP, F)
        im["m"] = mf[core * nb:(core + 1) * nb].reshape(nb * NP, 1)
        in_maps.append(im)
    results = run_cached(in_maps, nb, nch, ghconv_dtype, n_cores)
    out = np.zeros((Btot, NP, F), np.float32)
    for core in range(n_cores):
        for b in range(nb):
            out[core * nb + b] = results[core][f"out{b}"]
    return out, None


def kernel(**inputs):
    out, _ = run(inputs, nb=2, nch=100, n_cores=8)
    return out



# revision 9
# speedup vs baseline: 1.5502x; 1.5502x over previous
"""Trainium2 Bass kernel for nn_CombinedGraphLayer (LSH-binned GHConv message passing).

Contract: kernel(**inputs) takes FULL inputs (x [16,12800,256], msk [16,12800],
training scalar + weights), returns FULL output [16,12800,256].

Strategy: pure data-parallel over batch (2 batches per NeuronCore x 8 cores).
Per batch, on device:
  phase A  (per 128-point chunk): layernorm -> ffn_dist -> LSH argmax key;
           accumulate per-chunk key histograms; keep z*m / x_dist resident.
  phase A2: counting-sort ranks via prefix sums (matmul + scan tricks);
           indirect-scatter packed rows into sorted order (DRAM).
  phase B  (per 128-point bin): pairwise gaussian adjacency + 2 GHConv
           layers; indirect-scatter results back to original row order.

gamma/beta of the layernorm are folded into the ffn/GHConv weights on host
(pure input marshaling); msk is converted to float once on host.
"""

import numpy as np

import concourse.bass as bass
import concourse.tile as tile
from concourse import mybir
from concourse.bass_utils import run_bass_kernel_spmd
from concourse.masks import make_identity

dt = mybir.dt
OP = mybir.AluOpType
AF = mybir.ActivationFunctionType
IOA = bass.IndirectOffsetOnAxis

ABLATION = ""  # bench knob: "noB", "noscat", "nosort" (timing experiments only)

F = 256       # feature dim
D = 128       # distance dim
BIN = 128

# packed row layout (fp32): [ zm(0:256) | xd(256:384) | m(384) | idx(385) | pad ]
RW = 388
COL_M = 384
COL_IDX = 385


def split_excess_waits(nc):
    """This walrus build rejects instructions carrying more than a couple of
    sem waits (1 for CTRL-class like Drain, ~2 for compute). Move excess
    waits onto extra Drains inserted just before, on the same engine."""
    for f in nc.m.functions:
        for b in f.blocks:
            new_insts = []
            for inst in b.instructions:
                si = getattr(inst, "sync_info", None)
                ow = list(si.on_wait) if si is not None and si.on_wait else []
                limit = 1
                if len(ow) > limit and inst.engine is not None:
                    keep = ow[-limit:]
                    for w in ow[:-limit]:
                        d = mybir.InstNoOp(
                            name=nc.get_next_instruction_name(), ins=[], outs=[]
                        )
                        d.engine = inst.engine
                        d.sync_info = mybir.SyncInfo(on_wait=[w], on_update=[])
                        new_insts.append(d)
                    si.on_wait = keep
                new_insts.append(inst)
            b.instructions = new_insts


def build(nb, nch, ghconv_dtype=dt.float32, nbu=None):
    """Build the Bass module for nb batches of nch 128-point chunks each.

    nbu: number of sorted 128-row bins actually computed/emitted per batch
    (all unmasked rows sort into the first ~nch/2 bins; the tail is
    masked rows whose output is exactly zero)."""
    NP = nch * BIN            # points per batch
    NBINS = nch               # bins == chunks
    if nbu is None:
        nbu = NBINS
    CB = NBINS // 2           # used codebook columns
    NK = 2 * NBINS - 1        # distinct sort keys: 0 .. NK-1
    NKP = NK + 1              # padded width
    f32 = dt.float32
    bf16 = dt.bfloat16
    use_r = ghconv_dtype == dt.float32r
    gdt = ghconv_dtype

    nc = bass.Bass("TRN2", target_bir_lowering=False, debug=False)

    x_in = nc.dram_tensor("x", [nb * NP, F], f32, kind="ExternalInput").ap()
    m_in = nc.dram_tensor("m", [nb * NP, 1], f32, kind="ExternalInput").ap()
    wspec = [
        ("W1g", [F, D]), ("b1gb", [1, D]), ("W2", [D, D]), ("b2", [1, D]),
        ("CB", [D, CB]),
        ("th0", [F, F]), ("Wh0", [F, F]), ("Wt0", [F, F]),
        ("bth0", [1, F]), ("bhh0", [1, F]), ("bgt0", [1, F]),
        ("th1", [F, F]), ("Wh1", [F, F]), ("Wt1", [F, F]), ("bt1", [1, F]),
    ]
    wdram = {n: nc.dram_tensor(n, s, f32, kind="ExternalInput").ap() for n, s in wspec}
    outs = [nc.dram_tensor(f"out{b}", [nbu * BIN, F], bf16,
                           kind="ExternalOutput").ap()
            for b in range(nb)]
    oidx = [nc.dram_tensor(f"oidx{b}", [nbu * BIN, 1], f32,
                           kind="ExternalOutput").ap()
            for b in range(nb)]
    psort = [nc.dram_tensor(f"psort{b}", [NP, RW], f32, kind="Internal").ap()
             for b in range(nb)]

    with tile.TileContext(nc) as tc:
        with tc.tile_pool(name="init", bufs=1) as ip:
            ident = ip.tile([128, 128], f32)
            make_identity(nc, ident[:])
            eps_t = ip.tile([128, 1], f32)
            nc.vector.memset(eps_t[:], 1e-6)
            iota_p_i = ip.tile([128, 1], dt.int32)
            nc.gpsimd.iota(iota_p_i[:], [[0, 1]], base=0, channel_multiplier=1)
            iota_p_f = ip.tile([128, 1], f32)
            nc.vector.tensor_copy(iota_p_f[:], iota_p_i[:])
            iota_row_i = ip.tile([128, NKP], dt.int32)
            nc.gpsimd.iota(iota_row_i[:], [[1, NKP]], base=0, channel_multiplier=0)
            iota_row_f = ip.tile([128, NKP], f32)
            nc.vector.tensor_copy(iota_row_f[:], iota_row_i[:])
            # strictly-lower mask Tp[p', p] = 1 if p' < p  (for within-chunk cumsum)
            iota_r128_i = ip.tile([128, 128], dt.int32)
            nc.gpsimd.iota(iota_r128_i[:], [[1, 128]], base=0, channel_multiplier=0)
            iota_r128_f = ip.tile([128, 128], f32)
            nc.vector.tensor_copy(iota_r128_f[:], iota_r128_i[:])
            Tp_bf = ip.tile([128, 128], bf16)
            nc.vector.tensor_scalar(
                out=Tp_bf[:], in0=iota_r128_f[:], scalar1=iota_p_f[:],
                scalar2=None, op0=OP.is_gt)
            ones_col_bf = ip.tile([128, 1], bf16)
            nc.vector.memset(ones_col_bf[:], 1.0)
            ones_row_f = ip.tile([1, 128], f32)
            nc.vector.memset(ones_row_f[:], 1.0)
            ones_row_g = ip.tile([1, 128], gdt)
            if gdt == f32:
                nc.vector.memset(ones_row_g[:], 1.0)
            else:
                nc.vector.tensor_copy(ones_row_g[:], ones_row_f[:])

            # weights to SBUF
            wsb = {}
            for n, s in wspec:
                wdt = f32
                if n in ("th0", "Wh0", "Wt0", "th1", "Wh1", "Wt1",
                         "bth0", "bhh0", "bgt0", "bt1"):
                    wdt = gdt
                shp = ([128, s[0] // 128, s[1]] if s[0] > 128 else list(s))
                src = (wdram[n].rearrange("(c p) m -> p c m", p=128)
                       if s[0] > 128 else wdram[n][:])
                if wdt == f32:
                    t = ip.tile(shp, f32, tag=f"w_{n}")
                    nc.gpsimd.dma_start(out=t[:], in_=src)
                else:
                    stg = ip.tile(shp, f32, tag="w_stage")
                    nc.gpsimd.dma_start(out=stg[:], in_=src)
                    t = ip.tile(shp, wdt, tag=f"w_{n}")
                    nc.vector.tensor_copy(t[:], stg[:])
                wsb[n] = t

            for b in range(nb):
                _one_batch(tc, nc, b, nb, nch, NP, NBINS, CB, NK, NKP, nbu,
                           x_in, m_in, wsb, outs[b], oidx[b], psort[b],
                           ident, eps_t, iota_p_f, iota_row_f, Tp_bf,
                           ones_col_bf, ones_row_f, ones_row_g, gdt, use_r)

    split_excess_waits(nc)
    return nc


def _one_batch(tc, nc, b, nb, nch, NP, NBINS, CB, NK, NKP, nbu,
               x_in, m_in, wsb, out_d, oidx_d, psort_d,
               ident, eps_t, iota_p_f, iota_row_f, Tp_bf,
               ones_col_bf, ones_row_f, ones_row_g, gdt, use_r):
    f32 = dt.float32
    bf16 = dt.bfloat16
    KHI = float(NBINS - 1)
    if use_r:
        def R(ap):
            return ap.bitcast(dt.float32r)
    else:
        def R(ap):
            return ap

    with tc.tile_pool(name=f"res{b}", bufs=1) as rp, \
         tc.tile_pool(name=f"resps{b}", bufs=1, space="PSUM") as rpp:
        packed = rp.tile([128, nch, RW], f32)     # resident z*m / xd / m / idx
        key_all = rp.tile([128, nch], f32)
        rank_f = rp.tile([128, nch], f32)
        rank_u = rp.tile([128, nch], dt.uint32)
        T_lo = rpp.tile([NBINS, nch], f32, space="PSUM")
        T_hi = rpp.tile([NBINS - 1, nch], f32, space="PSUM")

        # ---------------- phase A ----------------
        with tc.tile_pool(name=f"pa{b}", bufs=3) as pa, \
             tc.tile_pool(name=f"paps{b}", bufs=1, space="PSUM") as pap:
            for c in range(nch):
                row0 = b * NP + c * 128
                x_t = pa.tile([128, F], f32)
                nc.sync.dma_start(out=x_t[:], in_=x_in[row0:row0 + 128, :])
                nc.sync.dma_start(out=packed[:, c, COL_M:COL_M + 1],
                                  in_=m_in[row0:row0 + 128, :])
                m_ap = packed[:, c, COL_M:COL_M + 1]

                st = pa.tile([128, 6], f32)
                nc.vector.bn_stats(out=st[:], in_=x_t[:])
                mv = pa.tile([128, 2], f32)
                nc.vector.bn_aggr(out=mv[:], in_=st[:])
                nc.scalar.activation(out=mv[:, 1:2], in_=mv[:, 1:2],
                                     func=AF.Sqrt, bias=eps_t[:])
                nc.vector.reciprocal(out=mv[:, 1:2], in_=mv[:, 1:2])
                z_t = pa.tile([128, F], f32)
                nc.vector.tensor_scalar(
                    out=z_t[:], in0=x_t[:], scalar1=mv[:, 0:1],
                    scalar2=mv[:, 1:2], op0=OP.subtract, op1=OP.mult)
                # zm into packed (gpsimd: SBUF only)
                nc.gpsimd.tensor_scalar_mul(packed[:, c, 0:F], z_t[:], m_ap)

                # zT (feature-major) for the ffn matmuls
                zT_ps = pap.tile([128, 2, 128], f32, space="PSUM")
                for k in range(2):
                    nc.tensor.transpose(zT_ps[:, k, :],
                                        z_t[:, k * 128:(k + 1) * 128], ident[:])
                zT_sb = pa.tile([128, 2, 128], f32)
                nc.scalar.activation(out=zT_sb[:], in_=zT_ps[:], func=AF.Copy)

                # hT = W1g^T zT + b1gb  (feature-major [D, pts])
                h_ps = pap.tile([128, 128], f32, space="PSUM")
                nc.tensor.matmul(h_ps[:], lhsT=wsb["W1g"][:, 0, :],
                                 rhs=zT_sb[:, 0, :], start=True, stop=False)
                nc.tensor.matmul(h_ps[:], lhsT=wsb["W1g"][:, 1, :],
                                 rhs=zT_sb[:, 1, :], start=False, stop=False)
                nc.tensor.matmul(h_ps[:], lhsT=wsb["b1gb"][:],
                                 rhs=ones_row_f[:], start=False, stop=True)
                # elu
                e_t = pa.tile([128, 128], f32)
                nc.vector.tensor_scalar_min(e_t[:], h_ps[:], 0.0)
                nc.scalar.activation(out=e_t[:], in_=e_t[:], func=AF.Exp)
                r_t = pa.tile([128, 128], f32)
                nc.scalar.activation(out=r_t[:], in_=h_ps[:], func=AF.Relu)
                hTe = pa.tile([128, 128], f32)
                nc.vector.scalar_tensor_tensor(
                    out=hTe[:], in0=e_t[:], scalar=-1.0, in1=r_t[:],
                    op0=OP.add, op1=OP.add)

                # xdT = W2^T hTe + b2
                xdT_ps = pap.tile([128, 128], f32, space="PSUM")
                nc.tensor.matmul(xdT_ps[:], lhsT=wsb["W2"][:], rhs=hTe[:],
                                 start=True, stop=False)
                nc.tensor.matmul(xdT_ps[:], lhsT=wsb["b2"][:],
                                 rhs=ones_row_f[:], start=False, stop=True)
                xdT_sb = pa.tile([128, 128], f32)
                nc.scalar.activation(out=xdT_sb[:], in_=xdT_ps[:], func=AF.Copy)
                # xd point-major into packed
                xd_ps = pap.tile([128, 128], f32, space="PSUM")
                nc.tensor.transpose(xd_ps[:], xdT_sb[:], ident[:])
                nc.vector.tensor_copy(packed[:, c, F:F + 128], xd_ps[:])
                # idx column
                nc.vector.tensor_scalar_add(
                    packed[:, c, COL_IDX:COL_IDX + 1], iota_p_f[:], float(c * 128))

                # mul = xd @ codebook  (point-major [pts, CB])
                mul_ps = pap.tile([128, CB], f32, space="PSUM")
                nc.tensor.matmul(mul_ps[:], lhsT=xdT_sb[:], rhs=wsb["CB"][:],
                                 start=True, stop=True)
                cmul = pa.tile([128, 2 * CB], f32)
                nc.scalar.activation(out=cmul[:, 0:CB], in_=mul_ps[:], func=AF.Copy)
                nc.scalar.activation(out=cmul[:, CB:2 * CB], in_=mul_ps[:],
                                     func=AF.Copy, scale=-1.0)
                mx8 = pa.tile([128, 8], f32)
                nc.vector.max(out=mx8[:], in_=cmul[:])
                ix8 = pa.tile([128, 8], dt.uint32)
                nc.vector.max_index(out=ix8[:], in_max=mx8[:], in_values=cmul[:])
                idxf = pa.tile([128, 1], f32)
                nc.vector.tensor_copy(idxf[:], ix8[:, 0:1])
                # key = argmax + (NBINS-1)*(1-m)
                nc.vector.scalar_tensor_tensor(
                    out=key_all[:, c:c + 1], in0=m_ap, scalar=-KHI, in1=idxf[:],
                    op0=OP.mult, op1=OP.add)
                nc.vector.tensor_scalar_add(key_all[:, c:c + 1],
                                            key_all[:, c:c + 1], KHI)
                # onehot -> per-chunk histogram columns
                oh = pa.tile([128, NKP], bf16)
                nc.vector.tensor_scalar(
                    out=oh[:], in0=iota_row_f[:, 0:NKP], scalar1=key_all[:, c:c + 1],
                    scalar2=None, op0=OP.is_equal)
                nc.tensor.matmul(T_lo[:, c:c + 1], lhsT=oh[:, 0:NBINS],
                                 rhs=ones_col_bf[:], start=True, stop=True)
                nc.tensor.matmul(T_hi[:, c:c + 1], lhsT=oh[:, NBINS:NK],
                                 rhs=ones_col_bf[:], start=True, stop=True)

        # ---------------- phase A2: ranks + sort scatter ----------------
        with tc.tile_pool(name=f"pa2{b}", bufs=2) as p2, \
             tc.tile_pool(name=f"pa2ps{b}", bufs=1, space="PSUM") as p2p:
            Tl_sb = p2.tile([NBINS, nch], f32)
            nc.scalar.activation(out=Tl_sb[:], in_=T_lo[:], func=AF.Copy)
            Th_sb = p2.tile([NBINS - 1, nch], f32)
            nc.scalar.activation(out=Th_sb[:], in_=T_hi[:], func=AF.Copy)
            # inclusive scan along chunks
            Sl_in = p2.tile([NBINS, nch], f32)
            nc.vector.tensor_tensor_scan(Sl_in[:], Tl_sb[:], Tl_sb[:], 0.0,
                                         OP.add, OP.bypass)
            Sh_in = p2.tile([NBINS - 1, nch], f32)
            nc.vector.tensor_tensor_scan(Sh_in[:], Th_sb[:], Th_sb[:], 0.0,
                                         OP.add, OP.bypass)
            # exclusive
            Sl_ex = p2.tile([NBINS, nch], f32)
            nc.vector.tensor_sub(Sl_ex[:], Sl_in[:], Tl_sb[:])
            Sh_ex = p2.tile([NBINS - 1, nch], f32)
            nc.vector.tensor_sub(Sh_ex[:], Sh_in[:], Th_sb[:])
            # grand totals -> key-offsets (exclusive cumsum over keys)
            grow_ps = p2p.tile([1, NKP], f32, space="PSUM")
            nc.tensor.transpose(grow_ps[:, 0:NBINS], Sl_in[:, nch - 1:nch],
                                ident[0:NBINS, 0:NBINS])
            nc.tensor.transpose(grow_ps[:, NBINS:NK], Sh_in[:, nch - 1:nch],
                                ident[0:NBINS - 1, 0:NBINS - 1])
            grow_sb = p2.tile([1, NKP], f32)
            nc.vector.memset(grow_sb[:], 0.0)
            nc.scalar.activation(out=grow_sb[:, 0:NK], in_=grow_ps[:, 0:NK],
                                 func=AF.Copy)
            ginc = p2.tile([1, NKP], f32)
            nc.vector.tensor_tensor_scan(ginc[:], grow_sb[:], grow_sb[:], 0.0,
                                         OP.add, OP.bypass)
            gexc = p2.tile([1, NKP], f32)
            nc.vector.tensor_sub(gexc[:], ginc[:], grow_sb[:])
            offs_ps = p2p.tile([NBINS, 2], f32, space="PSUM")
            nc.tensor.transpose(offs_ps[:, 0:1], gexc[:, 0:NBINS], ident[0:1, 0:1])
            nc.tensor.transpose(offs_ps[0:NBINS - 1, 1:2], gexc[:, NBINS:NK],
                                ident[0:1, 0:1])
            offs_sb = p2.tile([NBINS, 2], f32)
            nc.scalar.activation(out=offs_sb[:], in_=offs_ps[:], func=AF.Copy)
            nc.vector.tensor_scalar_add(Sl_ex[:], Sl_ex[:], offs_sb[:, 0:1])
            nc.vector.tensor_scalar_add(Sh_ex[:], Sh_ex[:],
                                        offs_sb[0:NBINS - 1, 1:2])
            # St[c, k] = base for chunk c / key k
            St_ps = p2p.tile([nch, NKP], f32, space="PSUM")
            nc.tensor.transpose(St_ps[:, 0:NBINS], Sl_ex[:],
                                ident[0:NBINS, 0:NBINS])
            nc.tensor.transpose(St_ps[:, NBINS:NK], Sh_ex[:],
                                ident[0:NBINS - 1, 0:NBINS - 1])
            St_sb = p2.tile([nch, NKP], f32)
            nc.vector.memset(St_sb[:], 0.0)
            nc.scalar.activation(out=St_sb[:, 0:NK], in_=St_ps[:, 0:NK],
                                 func=AF.Copy)

            for c in range(nch):
                oh2 = p2.tile([128, NKP], bf16)
                nc.vector.tensor_scalar(
                    out=oh2[:], in0=iota_row_f[:, 0:NKP],
                    scalar1=key_all[:, c:c + 1], scalar2=None, op0=OP.is_equal)
                St_row = p2.tile([1, NKP], f32)
                nc.sync.dma_start(out=St_row[:], in_=St_sb[c:c + 1, :])
                C_ps = p2p.tile([128, NKP], f32, space="PSUM")
                nc.tensor.matmul(C_ps[:], lhsT=Tp_bf[:], rhs=oh2[:],
                                 start=True, stop=False)
                nc.tensor.matmul(C_ps[:], lhsT=ones_row_f[:],
                                 rhs=St_row[:], start=False, stop=True)
                scr = p2.tile([128, NKP], f32)
                nc.vector.tensor_tensor(out=scr[:], in0=oh2[:], in1=C_ps[:],
                                        op=OP.mult)
                nc.vector.tensor_reduce(
                    out=rank_f[:, c:c + 1], in_=scr[:],
                    axis=mybir.AxisListType.X, op=OP.add)
            nc.vector.tensor_copy(rank_u[:], rank_f[:])
            for c in range(nch):
                if "nosort" in ABLATION:
                    nc.sync.dma_start(
                        out=psort_d[c * 128:(c + 1) * 128, :],
                        in_=packed[:, c, :])
                else:
                    nc.gpsimd.indirect_dma_start(
                        out=psort_d[:],
                        out_offset=IOA(ap=rank_u[:, c:c + 1], axis=0),
                        in_=packed[:, c, :], in_offset=None)

    # ---------------- phase B: adjacency + GHConv per bin ----------------
    if "noB" in ABLATION:
        return
    with tc.tile_pool(name=f"pb{b}", bufs=4) as pb, \
         tc.tile_pool(name=f"pbps{b}", bufs=1, space="PSUM") as pbp:
        for s in range(nbu):
            pk = pb.tile([128, RW], f32)
            nc.sync.dma_start(out=pk[:], in_=psort_d[s * 128:(s + 1) * 128, :])
            m_ap = pk[:, COL_M:COL_M + 1]
            # V cols: [na, one, one, na, m]; transposed pair/row tiles all
            # land at partition base 0 (matmul requires equal bases).
            V = pb.tile([128, 5], f32)
            sq = pb.tile([128, 128], f32)
            nc.scalar.activation(out=sq[:], in_=pk[:, F:F + 128],
                                 func=AF.Square, accum_out=V[:, 0:1])
            nc.gpsimd.memset(V[:, 1:3], 1.0)
            nc.gpsimd.tensor_copy(V[:, 3:4], V[:, 0:1])
            nc.gpsimd.tensor_copy(V[:, 4:5], m_ap)
            vt_ps = pbp.tile([2, 384], f32, space="PSUM")
            nc.tensor.transpose(vt_ps[0:2, 0:128], V[:, 0:2], ident[:])
            VTa = pb.tile([2, 128], f32)
            nc.scalar.activation(out=VTa[:], in_=vt_ps[0:2, 0:128],
                                 func=AF.Copy)
            nc.tensor.transpose(vt_ps[0:2, 128:256], V[:, 2:4], ident[:])
            VTb = pb.tile([2, 128], f32)
            nc.scalar.activation(out=VTb[:], in_=vt_ps[0:2, 128:256],
                                 func=AF.Copy)
            nc.tensor.transpose(vt_ps[0:1, 256:384], V[:, 4:5], ident[:])
            mT_sb = pb.tile([1, 128], f32)
            nc.scalar.activation(out=mT_sb[:], in_=vt_ps[0:1, 256:384],
                                 func=AF.Copy)
            # d2 = na_i - 2 xd xd^T + na_j ; M2 = m_i m_j
            adj_ps = pbp.tile([128, 384], f32, space="PSUM")
            xdT_ps = adj_ps[:, 0:128]
            d2_ps = adj_ps[:, 128:256]
            M2_ps = adj_ps[:, 256:384]
            nc.tensor.transpose(xdT_ps, pk[:, F:F + 128], ident[:])
            xdT = pb.tile([128, 128], f32)
            nc.scalar.activation(out=xdT[:], in_=xdT_ps, func=AF.Copy)
            xdTm2 = pb.tile([128, 128], f32)
            nc.scalar.activation(out=xdTm2[:], in_=xdT_ps, func=AF.Copy,
                                 scale=-2.0)
            nc.tensor.matmul(d2_ps, lhsT=xdTm2[:], rhs=xdT[:],
                             start=True, stop=False)
            nc.tensor.matmul(d2_ps, lhsT=VTa[:], rhs=VTb[:],
                             start=False, stop=True)
            nc.tensor.matmul(M2_ps, lhsT=mT_sb[:], rhs=mT_sb[:],
                             start=True, stop=True)
            dsc = pb.tile([128, 128], f32)
            nc.vector.tensor_scalar_max(dsc[:], d2_ps[:], 1e-6)
            nc.scalar.activation(out=dsc[:], in_=dsc[:], func=AF.Sqrt)
            nc.scalar.activation(out=dsc[:], in_=dsc[:], func=AF.Exp,
                                 scale=-0.1)
            dm = pb.tile([128, 128], gdt)
            ind = pb.tile([128, 1], f32)
            nc.vector.scalar_tensor_tensor(
                out=dm[:], in0=dsc[:], scalar=1.0, in1=M2_ps[:],
                op0=OP.mult, op1=OP.mult, accum_out=ind[:])
            nrm = pb.tile([128, 1], f32)
            nc.scalar.activation(out=nrm[:], in_=ind[:], func=AF.Sqrt,
                                 bias=eps_t[:])
            nc.vector.reciprocal(nrm[:], nrm[:])
            nc.vector.tensor_mul(nrm[:], nrm[:], m_ap)

            xb_ap = pk[:, 0:F]
            for li in range(2):
                sfx = "0" if li == 0 else "1"
                mm1 = pbp.tile([128, 512], f32, space="PSUM")
                mm2 = pbp.tile([128, 512], f32, space="PSUM")
                gat_ps = pbp.tile([128, F], f32, space="PSUM")
                xmT_ps = mm1[:, 0:256]
                hom2_ps = mm1[:, 256:512]
                hom_ps = mm2[:, 0:256]
                het_ps = mm2[:, 256:512]
                for k in range(2):
                    nc.tensor.transpose(
                        xmT_ps.rearrange("p (c q) -> p c q", q=128)[:, k, :],
                        xb_ap[:, k * 128:(k + 1) * 128], ident[:])
                xmT = pb.tile([128, 2, 128], gdt)
                nc.scalar.activation(out=xmT[:], in_=xmT_ps, func=AF.Copy)
                mT = mT_sb[:]
                if gdt != f32:
                    mTg = pb.tile([1, 128], gdt)
                    nc.vector.tensor_copy(mTg[:], mT_sb[:])
                    mT = mTg[:]
                # keep each PSUM accumulation group's matmuls consecutive
                for dst, wn, bias in (
                    (hom_ps, "th" + sfx, "bth0" if li == 0 else None),
                    (het_ps, "Wh" + sfx, "bhh0" if li == 0 else None),
                    (gat_ps[:], "Wt" + sfx,
                     "bgt0" if li == 0 else "bt1"),
                ):
                    for k in range(2):
                        nc.tensor.matmul(
                            dst, lhsT=R(xmT[:, k, :]), rhs=R(wsb[wn][:, k, :]),
                            start=(k == 0), stop=(k == 1 and bias is None))
                    if bias is not None:
                        blhs = mT if li == 0 else ones_row_g[:]
                        nc.tensor.matmul(dst, lhsT=R(blhs), rhs=R(wsb[bias][:]),
                                         start=False, stop=True)
                fh1 = pb.tile([128, F], gdt)
                nc.vector.tensor_scalar_mul(fh1[:], hom_ps[:], nrm[:])
                nc.tensor.matmul(hom2_ps[:], lhsT=R(dm[:]), rhs=R(fh1[:]),
                                 start=True, stop=True)
                gate = pb.tile([128, F], f32)
                nc.scalar.activation(out=gate[:], in_=gat_ps[:], func=AF.Sigmoid)
                fh2 = pb.tile([128, F], f32)
                nc.vector.tensor_scalar_mul(fh2[:], hom2_ps[:], nrm[:])
                nc.vector.tensor_sub(fh2[:], fh2[:], het_ps[:])
                nc.vector.tensor_mul(gate[:], gate[:], fh2[:])
                nc.vector.tensor_add(fh2[:], gate[:], het_ps[:])  # pre-act
                emin = pb.tile([128, F], f32)
                nc.gpsimd.tensor_scalar_min(emin[:], fh2[:], 0.0)
                nc.scalar.activation(out=emin[:], in_=emin[:], func=AF.Exp)
                er = pb.tile([128, F], f32)
                nc.scalar.activation(out=er[:], in_=fh2[:], func=AF.Relu)
                nc.vector.scalar_tensor_tensor(
                    out=emin[:], in0=emin[:], scalar=-1.0, in1=er[:],
                    op0=OP.add, op1=OP.add)
                out_t = pb.tile([128, F], f32)
                nc.gpsimd.tensor_scalar_mul(out_t[:], emin[:], m_ap)
                xb_ap = out_t[:]
            # emit sorted-order bf16 rows + original-index column; the host
            # scatters rows back to input order (tail bins are all-masked,
            # all-zero, and never emitted)
            obf = pb.tile([128, F], bf16)
            nc.vector.tensor_copy(obf[:], xb_ap)
            nc.sync.dma_start(out=out_d[s * 128:(s + 1) * 128, :], in_=obf[:])
            nc.sync.dma_start(out=oidx_d[s * 128:(s + 1) * 128, :],
                              in_=pk[:, COL_IDX:COL_IDX + 1])


def _fold_weights(inputs):
    g = inputs["ln_gamma"].astype(np.float32)
    be = inputs["ln_beta"].astype(np.float32)
    W1 = inputs["W1"].astype(np.float32)
    b1 = inputs["b1"].astype(np.float32)
    w = {
        "W1g": g[:, None] * W1,
        "b1gb": (b1 + be @ W1)[None, :],
        "W2": inputs["W2"].astype(np.float32),
        "b2": inputs["b2"].astype(np.float32)[None, :],
        "th1": inputs["th1"].astype(np.float32),
        "Wh1": inputs["Wh1"].astype(np.float32),
        "Wt1": inputs["Wt1"].astype(np.float32),
        "bt1": inputs["bt1"].astype(np.float32)[None, :],
    }
    for nm in ("th0", "Wh0", "Wt0"):
        w[nm] = g[:, None] * inputs[nm].astype(np.float32)
    w["bth0"] = (be @ inputs["th0"].astype(np.float32))[None, :]
    w["bhh0"] = (be @ inputs["Wh0"].astype(np.float32))[None, :]
    w["bgt0"] = (inputs["bt0"].astype(np.float32) +
                 be @ inputs["Wt0"].astype(np.float32))[None, :]
    return {k: np.ascontiguousarray(v, dtype=np.float32) for k, v in w.items()}


_BUILD_CACHE = {}


def _get_nc(nb, nch, ghconv_dtype=dt.float32, nbu=None):
    key = (nb, nch, ghconv_dtype, nbu, ABLATION)
    if key not in _BUILD_CACHE:
        _BUILD_CACHE[key] = build(nb, nch, ghconv_dtype, nbu=nbu)
    return _BUILD_CACHE[key]


_RUNNER_CACHE = {}


def _get_runner(nb, nch, ghconv_dtype, n_cores, nbu=None):
    """Cached jitted SPMD executor (re-jitting per call costs seconds)."""
    key = (nb, nch, ghconv_dtype, n_cores, nbu)
    if key in _RUNNER_CACHE:
        return _RUNNER_CACHE[key]
    import jax
    from jax.sharding import Mesh, PartitionSpec
    from jax.experimental.shard_map import shard_map
    from concourse import bass2jax

    bass2jax.install_neuronx_cc_hook()
    nc = _get_nc(nb, nch, ghconv_dtype, nbu=nbu)
    partition_name = (nc.partition_id_tensor.name
                      if nc.partition_id_tensor else None)
    in_names, out_names, out_avals, zero_shapes = [], [], [], []
    for alloc in nc.m.functions[0].allocations:
        if not isinstance(alloc, mybir.MemoryLocationSet):
            continue
        name = alloc.memorylocations[0].name
        if alloc.kind == "ExternalInput":
            if name != partition_name:
                in_names.append(name)
        elif alloc.kind == "ExternalOutput":
            out_names.append(name)
            shape = tuple(alloc.tensor_shape)
            dtype = mybir.dt.np(alloc.dtype)
            out_avals.append(jax.core.ShapedArray(shape, dtype))
            zero_shapes.append((shape, dtype))
    n_params = len(in_names)
    all_names = in_names + out_names
    if partition_name is not None:
        all_names = all_names + [partition_name]
    def _body(*args):
        operands = list(args)
        if partition_name is not None:
            operands.append(bass2jax.partition_id_tensor())
        outs = bass2jax._bass_exec_p.bind(
            *operands,
            out_avals=tuple(out_avals),
            in_names=tuple(all_names),
            out_names=tuple(out_names),
            lowering_input_output_aliases=(),
            sim_require_finite=True,
            sim_require_nnan=True,
            nc=nc,
        )
        return tuple(outs)

    devices = jax.devices()[:n_cores]
    mesh = Mesh(np.asarray(devices), ("core",))
    in_specs = (PartitionSpec("core"),) * (n_params + len(out_names))
    out_specs = (PartitionSpec("core"),) * len(out_names)
    sharded = jax.jit(
        shard_map(_body, mesh=mesh, in_specs=in_specs, out_specs=out_specs,
                  check_rep=False),
        keep_unused=True)
    # zero output buffers staged on device ONCE and reused read-only
    from jax.sharding import NamedSharding
    shard = NamedSharding(mesh, PartitionSpec("core"))
    dev_zeros = [
        jax.device_put(np.zeros((n_cores * s0[0], *s0[1:]), d), shard)
        for s0, d in zero_shapes]
    runner = (sharded, in_names, out_names, out_avals, dev_zeros)
    _RUNNER_CACHE[key] = runner
    return runner


NBU = 54  # sorted bins computed per batch; all unmasked rows land in the
          # first ~nch/2+1 bins (msk ~ Bernoulli(0.5)); runtime-verified.


def run(inputs, nb, nch, n_cores, ghconv_dtype=dt.float32, trace=False,
        nbu=NBU):
    """inputs: dict with x [Btot, NP, F] float32, msk [Btot, NP] bool + weights.
    Btot must equal n_cores * nb."""
    import concurrent.futures as cf
    NP = nch * BIN
    x = np.ascontiguousarray(inputs["x"], dtype=np.float32)
    msk = np.asarray(inputs["msk"])
    mf = np.ascontiguousarray(msk, dtype=np.float32)[..., None]
    Btot = x.shape[0]
    assert Btot == n_cores * nb
    w = _fold_weights(inputs)
    w["CB"] = np.ascontiguousarray(
        inputs["codebook"][:, :NP // BIN // 2], dtype=np.float32)

    sharded, in_names, out_names, out_avals, dev_zeros = _get_runner(
        nb, nch, ghconv_dtype, n_cores, nbu=nbu)
    # concatenated-over-cores layout == the full array reshaped (zero-copy
    # views for the big tensors)
    full = dict(w)
    full["x"] = x.reshape(Btot * NP, F)
    full["m"] = mf.reshape(Btot * NP, 1)
    concat_in = []
    for n in in_names:
        a = full[n]
        if n in ("x", "m"):
            concat_in.append(a)
        else:
            concat_in.append(np.broadcast_to(
                a[None], (n_cores, *a.shape)).reshape(n_cores * a.shape[0],
                                                      *a.shape[1:]))
    out_arrs = sharded(*concat_in, *dev_zeros)

    # fetch over a small thread pool (parallel streams are ~1.4x the
    # serialized tunnel rate)
    with cf.ThreadPoolExecutor(len(out_arrs)) as ex:
        host_outs = list(ex.map(np.asarray, out_arrs))
    res = dict(zip(out_names, host_outs))

    out = np.zeros((Btot, NP, F), np.float32)
    for core in range(n_cores):
        for b in range(nb):
            gb = core * nb + b
            vals = res[f"out{b}"].reshape(n_cores, nbu * BIN, F)[core]
            ids = res[f"oidx{b}"].reshape(n_cores, nbu * BIN)[core]
            ids = ids.astype(np.int64)
            out[gb, ids] = vals.astype(np.float32)
            # every unmasked row must have been emitted within the prefix
            covered = np.zeros(NP, bool)
            covered[ids] = True
            if not (covered | ~msk[gb]).all():
                raise RuntimeError(
                    f"batch {gb}: unmasked rows beyond {nbu} sorted bins; "
                    f"increase NBU")
    return out, None


def kernel(**inputs):
    out, _ = run(inputs, nb=2, nch=100, n_cores=8)
    return out



# revision 10
# speedup vs baseline: 2.4320x; 1.5688x over previous
"""Trainium2 Bass kernel for nn_CombinedGraphLayer (LSH-binned GHConv message passing).

Contract: kernel(**inputs) takes FULL inputs (x [16,12800,256], msk [16,12800],
training scalar + weights), returns FULL output [16,12800,256].

Strategy: pure data-parallel over batch (2 batches per NeuronCore x 8 cores).
The wall clock is dominated by the ~55-75 MB/s host<->device tunnel, so the
pipeline is organized around minimizing wire bytes:

  put   x as bf16 (105MB instead of 210MB fp32)
  modK  (device): LSH argmax + top-2 gap per row from the bf16 input
  host  rows whose gap < TAU could have a different argmax than the fp32
        reference chain; recompute those exactly in fp64-free numpy fp32
        (~10% of rows, ~0.2s), then argsort -> exact per-row sort ranks
  modM  (device): layernorm -> ffn_dist -> pack, indirect-scatter rows into
        sorted bin order using the host ranks, then per 128-point bin:
        pairwise gaussian adjacency + 2 GHConv layers. Only the first NBU
        sorted bins are computed/emitted (all unmasked rows sort there);
        output rows leave in sorted order as bf16 + original-index column
  host  scatter rows back to input order (bf16 output quantization adds
        ~2e-3 max-rel error vs the 2e-2 tolerance)

Weights are folded (layernorm gamma/beta into the ffn/GHConv weights) and
embedded in the NEFF as constants - zero per-call wire cost.
"""

import hashlib
import numpy as np

import concourse.bass as bass
import concourse.tile as tile
from concourse import mybir
from concourse.masks import make_identity

dt = mybir.dt
OP = mybir.AluOpType
AF = mybir.ActivationFunctionType
IOA = bass.IndirectOffsetOnAxis

ABLATION = ""  # bench knob: "noB" (timing experiments only)

F = 256       # feature dim
D = 128       # distance dim
BIN = 128

# packed row layout (fp32): [ zm(0:256) | xd(256:384) | m(384) | idx(385) | pad ]
RW = 388
COL_M = 384
COL_IDX = 385

NBU = 54   # sorted 128-row bins computed per batch; all unmasked rows land in
           # the first ~nch/2+1 bins (msk ~ Bernoulli(0.5)); runtime-verified.
TAU = 1e-2  # risky-gap threshold; max |cmul(bf16 x) - cmul(fp32 x)| measured
            # at 2.2e-3, so 1e-2 leaves >4x margin. ~10% of rows get an exact
            # host recompute of their LSH argmax.


def split_excess_waits(nc):
    """This walrus build rejects instructions carrying more than a couple of
    sem waits (1 for CTRL-class like Drain, ~2 for compute). Move excess
    waits onto extra Drains inserted just before, on the same engine."""
    for f in nc.m.functions:
        for b in f.blocks:
            new_insts = []
            for inst in b.instructions:
                si = getattr(inst, "sync_info", None)
                ow = list(si.on_wait) if si is not None and si.on_wait else []
                limit = 1
                if len(ow) > limit and inst.engine is not None:
                    keep = ow[-limit:]
                    for w in ow[:-limit]:
                        d = mybir.InstNoOp(
                            name=nc.get_next_instruction_name(), ins=[], outs=[]
                        )
                        d.engine = inst.engine
                        d.sync_info = mybir.SyncInfo(on_wait=[w], on_update=[])
                        new_insts.append(d)
                    si.on_wait = keep
                new_insts.append(inst)
            b.instructions = new_insts


def _ffn_head(nc, pa, pap, wsb, ident, eps_t, ones_row_f, xb_chunk, m_chunk=None):
    """Shared LN -> ffn_dist chain for one 128-row chunk of bf16 input.
    Returns (z_t fp32 [128,F], xdT_sb fp32 [128,128] feature-major)."""
    f32 = dt.float32
    xb_t = pa.tile([128, F], dt.bfloat16)
    nc.sync.dma_start(out=xb_t[:], in_=xb_chunk)
    x_t = pa.tile([128, F], f32)
    nc.vector.tensor_copy(x_t[:], xb_t[:])

    st = pa.tile([128, 6], f32)
    nc.vector.bn_stats(out=st[:], in_=x_t[:])
    mv = pa.tile([128, 2], f32)
    nc.vector.bn_aggr(out=mv[:], in_=st[:])
    nc.scalar.activation(out=mv[:, 1:2], in_=mv[:, 1:2],
                         func=AF.Sqrt, bias=eps_t[:])
    nc.vector.reciprocal(out=mv[:, 1:2], in_=mv[:, 1:2])
    z_t = pa.tile([128, F], f32)
    nc.vector.tensor_scalar(
        out=z_t[:], in0=x_t[:], scalar1=mv[:, 0:1],
        scalar2=mv[:, 1:2], op0=OP.subtract, op1=OP.mult)

    # zT (feature-major) for the ffn matmuls
    zT_ps = pap.tile([128, 2, 128], f32, space="PSUM")
    for k in range(2):
        nc.tensor.transpose(zT_ps[:, k, :],
                            z_t[:, k * 128:(k + 1) * 128], ident[:])
    zT_sb = pa.tile([128, 2, 128], f32)
    nc.scalar.activation(out=zT_sb[:], in_=zT_ps[:], func=AF.Copy)

    # hT = W1g^T zT + b1gb  (feature-major [D, pts])
    h_ps = pap.tile([128, 128], f32, space="PSUM")
    nc.tensor.matmul(h_ps[:], lhsT=wsb["W1g"][:, 0, :],
                     rhs=zT_sb[:, 0, :], start=True, stop=False)
    nc.tensor.matmul(h_ps[:], lhsT=wsb["W1g"][:, 1, :],
                     rhs=zT_sb[:, 1, :], start=False, stop=False)
    nc.tensor.matmul(h_ps[:], lhsT=wsb["b1gb"][:],
                     rhs=ones_row_f[:], start=False, stop=True)
    # elu
    e_t = pa.tile([128, 128], f32)
    nc.vector.tensor_scalar_min(e_t[:], h_ps[:], 0.0)
    nc.scalar.activation(out=e_t[:], in_=e_t[:], func=AF.Exp)
    r_t = pa.tile([128, 128], f32)
    nc.scalar.activation(out=r_t[:], in_=h_ps[:], func=AF.Relu)
    hTe = pa.tile([128, 128], f32)
    nc.vector.scalar_tensor_tensor(
        out=hTe[:], in0=e_t[:], scalar=-1.0, in1=r_t[:],
        op0=OP.add, op1=OP.add)

    # xdT = W2^T hTe + b2
    xdT_ps = pap.tile([128, 128], f32, space="PSUM")
    nc.tensor.matmul(xdT_ps[:], lhsT=wsb["W2"][:], rhs=hTe[:],
                     start=True, stop=False)
    nc.tensor.matmul(xdT_ps[:], lhsT=wsb["b2"][:],
                     rhs=ones_row_f[:], start=False, stop=True)
    xdT_sb = pa.tile([128, 128], f32)
    nc.scalar.activation(out=xdT_sb[:], in_=xdT_ps[:], func=AF.Copy)
    return z_t, xdT_sb


def build_keys(nb, nch, w):
    """modK: per-row LSH argmax + top-2 gap from bf16 input."""
    NP = nch * BIN
    CB = nch // 2
    f32 = dt.float32
    nc = bass.Bass("TRN2", target_bir_lowering=False, debug=False)

    x_in = nc.dram_tensor("x", [nb * NP, F], dt.bfloat16,
                          kind="ExternalInput").ap()
    am_d = nc.dram_tensor("am", [nb * NP, 1], f32, kind="ExternalOutput").ap()
    gap_d = nc.dram_tensor("gap", [nb * NP, 1], f32, kind="ExternalOutput").ap()
    wdram = {n: nc.inline_tensor(w[n], name=n).ap()
             for n in ("W1g", "b1gb", "W2", "b2", "CB")}

    with tile.TileContext(nc) as tc:
        with tc.tile_pool(name="init", bufs=1) as ip:
            ident = ip.tile([128, 128], f32)
            make_identity(nc, ident[:])
            eps_t = ip.tile([128, 1], f32)
            nc.vector.memset(eps_t[:], 1e-6)
            ones_row_f = ip.tile([1, 128], f32)
            nc.vector.memset(ones_row_f[:], 1.0)
            wsb = {}
            for n in ("W1g", "b1gb", "W2", "b2", "CB"):
                s = list(w[n].shape)
                shp = [128, s[0] // 128, s[1]] if s[0] > 128 else s
                src = (wdram[n].rearrange("(c p) m -> p c m", p=128)
                       if s[0] > 128 else wdram[n][:])
                t = ip.tile(shp, f32, tag=f"w_{n}")
                nc.gpsimd.dma_start(out=t[:], in_=src)
                wsb[n] = t

            with tc.tile_pool(name="pk", bufs=3) as pa, \
                 tc.tile_pool(name="pkps", bufs=1, space="PSUM") as pap:
                for c in range(nb * nch):
                    row0 = c * 128
                    _, xdT_sb = _ffn_head(nc, pa, pap, wsb, ident, eps_t,
                                          ones_row_f,
                                          x_in[row0:row0 + 128, :])
                    # mul = xd @ codebook  (point-major [pts, CB])
                    mul_ps = pap.tile([128, CB], f32, space="PSUM")
                    nc.tensor.matmul(mul_ps[:], lhsT=xdT_sb[:], rhs=wsb["CB"][:],
                                     start=True, stop=True)
                    cmul = pa.tile([128, 2 * CB], f32)
                    nc.scalar.activation(out=cmul[:, 0:CB], in_=mul_ps[:],
                                         func=AF.Copy)
                    nc.scalar.activation(out=cmul[:, CB:2 * CB], in_=mul_ps[:],
                                         func=AF.Copy, scale=-1.0)
                    mx8 = pa.tile([128, 8], f32)
                    nc.vector.max(out=mx8[:], in_=cmul[:])
                    ix8 = pa.tile([128, 8], dt.uint32)
                    nc.vector.max_index(out=ix8[:], in_max=mx8[:],
                                        in_values=cmul[:])
                    idxf = pa.tile([128, 1], f32)
                    nc.vector.tensor_copy(idxf[:], ix8[:, 0:1])
                    # top-2 gap: mask out max positions, re-reduce
                    mxv = pa.tile([128, 1], f32)
                    nc.vector.tensor_reduce(out=mxv[:], in_=cmul[:],
                                            axis=mybir.AxisListType.X,
                                            op=OP.max)
                    eq = pa.tile([128, 2 * CB], f32)
                    nc.vector.tensor_scalar(
                        out=eq[:], in0=cmul[:], scalar1=mxv[:],
                        scalar2=None, op0=OP.is_equal)
                    c2 = pa.tile([128, 2 * CB], f32)
                    nc.vector.scalar_tensor_tensor(
                        out=c2[:], in0=eq[:], scalar=-1e30, in1=cmul[:],
                        op0=OP.mult, op1=OP.add)
                    mx2 = pa.tile([128, 1], f32)
                    nc.vector.tensor_reduce(out=mx2[:], in_=c2[:],
                                            axis=mybir.AxisListType.X,
                                            op=OP.max)
                    gap_t = pa.tile([128, 1], f32)
                    nc.vector.tensor_sub(gap_t[:], mxv[:], mx2[:])
                    nc.sync.dma_start(out=am_d[row0:row0 + 128, :], in_=idxf[:])
                    nc.sync.dma_start(out=gap_d[row0:row0 + 128, :],
                                      in_=gap_t[:])

    split_excess_waits(nc)
    return nc


def build(nb, nch, w, ghconv_dtype=dt.float32, nbu=None):
    """modM: full pipeline given host-computed sort ranks; bf16 x input."""
    NP = nch * BIN
    NBINS = nch
    if nbu is None:
        nbu = NBINS
    f32 = dt.float32
    bf16 = dt.bfloat16
    use_r = ghconv_dtype == dt.float32r
    gdt = ghconv_dtype

    nc = bass.Bass("TRN2", target_bir_lowering=False, debug=False)

    x_in = nc.dram_tensor("x", [nb * NP, F], bf16, kind="ExternalInput").ap()
    m_in = nc.dram_tensor("m", [nb * NP, 1], f32, kind="ExternalInput").ap()
    rank_in = nc.dram_tensor("rank", [nb * NP, 1], dt.uint32,
                             kind="ExternalInput").ap()
    wnames = ["W1g", "b1gb", "W2", "b2",
              "th0", "Wh0", "Wt0", "bth0", "bhh0", "bgt0",
              "th1", "Wh1", "Wt1", "bt1"]
    wdram = {n: nc.inline_tensor(w[n], name=n).ap() for n in wnames}
    outs = [nc.dram_tensor(f"out{b}", [nbu * BIN, F], bf16,
                           kind="ExternalOutput").ap()
            for b in range(nb)]
    oidx = [nc.dram_tensor(f"oidx{b}", [nbu * BIN, 1], f32,
                           kind="ExternalOutput").ap()
            for b in range(nb)]
    psort = [nc.dram_tensor(f"psort{b}", [NP, RW], f32, kind="Internal").ap()
             for b in range(nb)]

    with tile.TileContext(nc) as tc:
        with tc.tile_pool(name="init", bufs=1) as ip:
            ident = ip.tile([128, 128], f32)
            make_identity(nc, ident[:])
            eps_t = ip.tile([128, 1], f32)
            nc.vector.memset(eps_t[:], 1e-6)
            iota_p_i = ip.tile([128, 1], dt.int32)
            nc.gpsimd.iota(iota_p_i[:], [[0, 1]], base=0, channel_multiplier=1)
            iota_p_f = ip.tile([128, 1], f32)
            nc.vector.tensor_copy(iota_p_f[:], iota_p_i[:])
            ones_row_f = ip.tile([1, 128], f32)
            nc.vector.memset(ones_row_f[:], 1.0)
            ones_row_g = ip.tile([1, 128], gdt)
            if gdt == f32:
                nc.vector.memset(ones_row_g[:], 1.0)
            else:
                nc.vector.tensor_copy(ones_row_g[:], ones_row_f[:])

            # weights to SBUF
            wsb = {}
            for n in wnames:
                s = list(w[n].shape)
                wdt = f32
                if n in ("th0", "Wh0", "Wt0", "th1", "Wh1", "Wt1",
                         "bth0", "bhh0", "bgt0", "bt1"):
                    wdt = gdt
                shp = [128, s[0] // 128, s[1]] if s[0] > 128 else s
                src = (wdram[n].rearrange("(c p) m -> p c m", p=128)
                       if s[0] > 128 else wdram[n][:])
                if wdt == f32:
                    t = ip.tile(shp, f32, tag=f"w_{n}")
                    nc.gpsimd.dma_start(out=t[:], in_=src)
                else:
                    stg = ip.tile(shp, f32, tag="w_stage")
                    nc.gpsimd.dma_start(out=stg[:], in_=src)
                    t = ip.tile(shp, wdt, tag=f"w_{n}")
                    nc.vector.tensor_copy(t[:], stg[:])
                wsb[n] = t

            for b in range(nb):
                _one_batch(tc, nc, b, nb, nch, NP, NBINS, nbu,
                           x_in, m_in, rank_in, wsb, outs[b], oidx[b],
                           psort[b], ident, eps_t, iota_p_f,
                           ones_row_f, ones_row_g, gdt, use_r)

    split_excess_waits(nc)
    return nc


def _one_batch(tc, nc, b, nb, nch, NP, NBINS, nbu,
               x_in, m_in, rank_in, wsb, out_d, oidx_d, psort_d,
               ident, eps_t, iota_p_f, ones_row_f, ones_row_g, gdt, use_r):
    f32 = dt.float32
    bf16 = dt.bfloat16
    if use_r:
        def R(ap):
            return ap.bitcast(dt.float32r)
    else:
        def R(ap):
            return ap

    with tc.tile_pool(name=f"res{b}", bufs=1) as rp:
        packed = rp.tile([128, nch, RW], f32)     # resident z*m / xd / m / idx
        rank_u = rp.tile([128, nch], dt.uint32)

        # ---------------- phase A: LN -> ffn -> pack ----------------
        with tc.tile_pool(name=f"pa{b}", bufs=3) as pa, \
             tc.tile_pool(name=f"paps{b}", bufs=1, space="PSUM") as pap:
            for c in range(nch):
                row0 = b * NP + c * 128
                nc.sync.dma_start(out=packed[:, c, COL_M:COL_M + 1],
                                  in_=m_in[row0:row0 + 128, :])
                nc.sync.dma_start(out=rank_u[:, c:c + 1],
                                  in_=rank_in[row0:row0 + 128, :])
                m_ap = packed[:, c, COL_M:COL_M + 1]
                z_t, xdT_sb = _ffn_head(nc, pa, pap, wsb, ident, eps_t,
                                        ones_row_f, x_in[row0:row0 + 128, :])
                # zm into packed (gpsimd: SBUF only)
                nc.gpsimd.tensor_scalar_mul(packed[:, c, 0:F], z_t[:], m_ap)
                # xd point-major into packed
                xd_ps = pap.tile([128, 128], f32, space="PSUM")
                nc.tensor.transpose(xd_ps[:], xdT_sb[:], ident[:])
                nc.vector.tensor_copy(packed[:, c, F:F + 128], xd_ps[:])
                # idx column
                nc.vector.tensor_scalar_add(
                    packed[:, c, COL_IDX:COL_IDX + 1], iota_p_f[:],
                    float(c * 128))

        # ---------------- phase A2: scatter rows to sorted order ----------
        for c in range(nch):
            nc.gpsimd.indirect_dma_start(
                out=psort_d[:],
                out_offset=IOA(ap=rank_u[:, c:c + 1], axis=0),
                in_=packed[:, c, :], in_offset=None)

    # ---------------- phase B: adjacency + GHConv per bin ----------------
    if "noB" in ABLATION:
        return
    with tc.tile_pool(name=f"pb{b}", bufs=4) as pb, \
         tc.tile_pool(name=f"pbps{b}", bufs=1, space="PSUM") as pbp:
        for s in range(nbu):
            pk = pb.tile([128, RW], f32)
            nc.sync.dma_start(out=pk[:], in_=psort_d[s * 128:(s + 1) * 128, :])
            m_ap = pk[:, COL_M:COL_M + 1]
            # V cols: [na, one, one, na, m]; transposed pair/row tiles all
            # land at partition base 0 (matmul requires equal bases).
            V = pb.tile([128, 5], f32)
            sq = pb.tile([128, 128], f32)
            nc.scalar.activation(out=sq[:], in_=pk[:, F:F + 128],
                                 func=AF.Square, accum_out=V[:, 0:1])
            nc.gpsimd.memset(V[:, 1:3], 1.0)
            nc.gpsimd.tensor_copy(V[:, 3:4], V[:, 0:1])
            nc.gpsimd.tensor_copy(V[:, 4:5], m_ap)
            vt_ps = pbp.tile([2, 384], f32, space="PSUM")
            nc.tensor.transpose(vt_ps[0:2, 0:128], V[:, 0:2], ident[:])
            VTa = pb.tile([2, 128], f32)
            nc.scalar.activation(out=VTa[:], in_=vt_ps[0:2, 0:128],
                                 func=AF.Copy)
            nc.tensor.transpose(vt_ps[0:2, 128:256], V[:, 2:4], ident[:])
            VTb = pb.tile([2, 128], f32)
            nc.scalar.activation(out=VTb[:], in_=vt_ps[0:2, 128:256],
                                 func=AF.Copy)
            nc.tensor.transpose(vt_ps[0:1, 256:384], V[:, 4:5], ident[:])
            mT_sb = pb.tile([1, 128], f32)
            nc.scalar.activation(out=mT_sb[:], in_=vt_ps[0:1, 256:384],
                                 func=AF.Copy)
            # d2 = na_i - 2 xd xd^T + na_j ; M2 = m_i m_j
            adj_ps = pbp.tile([128, 384], f32, space="PSUM")
            xdT_ps = adj_ps[:, 0:128]
            d2_ps = adj_ps[:, 128:256]
            M2_ps = adj_ps[:, 256:384]
            nc.tensor.transpose(xdT_ps, pk[:, F:F + 128], ident[:])
            xdT = pb.tile([128, 128], f32)
            nc.scalar.activation(out=xdT[:], in_=xdT_ps, func=AF.Copy)
            xdTm2 = pb.tile([128, 128], f32)
            nc.scalar.activation(out=xdTm2[:], in_=xdT_ps, func=AF.Copy,
                                 scale=-2.0)
            nc.tensor.matmul(d2_ps, lhsT=xdTm2[:], rhs=xdT[:],
                             start=True, stop=False)
            nc.tensor.matmul(d2_ps, lhsT=VTa[:], rhs=VTb[:],
                             start=False, stop=True)
            nc.tensor.matmul(M2_ps, lhsT=mT_sb[:], rhs=mT_sb[:],
                             start=True, stop=True)
            dsc = pb.tile([128, 128], f32)
            nc.vector.tensor_scalar_max(dsc[:], d2_ps[:], 1e-6)
            nc.scalar.activation(out=dsc[:], in_=dsc[:], func=AF.Sqrt)
            nc.scalar.activation(out=dsc[:], in_=dsc[:], func=AF.Exp,
                                 scale=-0.1)
            dm = pb.tile([128, 128], gdt)
            ind = pb.tile([128, 1], f32)
            nc.vector.scalar_tensor_tensor(
                out=dm[:], in0=dsc[:], scalar=1.0, in1=M2_ps[:],
                op0=OP.mult, op1=OP.mult, accum_out=ind[:])
            nrm = pb.tile([128, 1], f32)
            nc.scalar.activation(out=nrm[:], in_=ind[:], func=AF.Sqrt,
                                 bias=eps_t[:])
            nc.vector.reciprocal(nrm[:], nrm[:])
            nc.vector.tensor_mul(nrm[:], nrm[:], m_ap)

            xb_ap = pk[:, 0:F]
            for li in range(2):
                sfx = "0" if li == 0 else "1"
                mm1 = pbp.tile([128, 512], f32, space="PSUM")
                mm2 = pbp.tile([128, 512], f32, space="PSUM")
                gat_ps = pbp.tile([128, F], f32, space="PSUM")
                xmT_ps = mm1[:, 0:256]
                hom2_ps = mm1[:, 256:512]
                hom_ps = mm2[:, 0:256]
                het_ps = mm2[:, 256:512]
                for k in range(2):
                    nc.tensor.transpose(
                        xmT_ps.rearrange("p (c q) -> p c q", q=128)[:, k, :],
                        xb_ap[:, k * 128:(k + 1) * 128], ident[:])
                xmT = pb.tile([128, 2, 128], gdt)
                nc.scalar.activation(out=xmT[:], in_=xmT_ps, func=AF.Copy)
                mT = mT_sb[:]
                if gdt != f32:
                    mTg = pb.tile([1, 128], gdt)
                    nc.vector.tensor_copy(mTg[:], mT_sb[:])
                    mT = mTg[:]
                # keep each PSUM accumulation group's matmuls consecutive
                for dst, wn, bias in (
                    (hom_ps, "th" + sfx, "bth0" if li == 0 else None),
                    (het_ps, "Wh" + sfx, "bhh0" if li == 0 else None),
                    (gat_ps[:], "Wt" + sfx,
                     "bgt0" if li == 0 else "bt1"),
                ):
                    for k in range(2):
                        nc.tensor.matmul(
                            dst, lhsT=R(xmT[:, k, :]), rhs=R(wsb[wn][:, k, :]),
                            start=(k == 0), stop=(k == 1 and bias is None))
                    if bias is not None:
                        blhs = mT if li == 0 else ones_row_g[:]
                        nc.tensor.matmul(dst, lhsT=R(blhs), rhs=R(wsb[bias][:]),
                                         start=False, stop=True)
                fh1 = pb.tile([128, F], gdt)
                nc.vector.tensor_scalar_mul(fh1[:], hom_ps[:], nrm[:])
                nc.tensor.matmul(hom2_ps[:], lhsT=R(dm[:]), rhs=R(fh1[:]),
                                 start=True, stop=True)
                gate = pb.tile([128, F], f32)
                nc.scalar.activation(out=gate[:], in_=gat_ps[:], func=AF.Sigmoid)
                fh2 = pb.tile([128, F], f32)
                nc.vector.tensor_scalar_mul(fh2[:], hom2_ps[:], nrm[:])
                nc.vector.tensor_sub(fh2[:], fh2[:], het_ps[:])
                nc.vector.tensor_mul(gate[:], gate[:], fh2[:])
                nc.vector.tensor_add(fh2[:], gate[:], het_ps[:])  # pre-act
                emin = pb.tile([128, F], f32)
                nc.gpsimd.tensor_scalar_min(emin[:], fh2[:], 0.0)
                nc.scalar.activation(out=emin[:], in_=emin[:], func=AF.Exp)
                er = pb.tile([128, F], f32)
                nc.scalar.activation(out=er[:], in_=fh2[:], func=AF.Relu)
                nc.vector.scalar_tensor_tensor(
                    out=emin[:], in0=emin[:], scalar=-1.0, in1=er[:],
                    op0=OP.add, op1=OP.add)
                out_t = pb.tile([128, F], f32)
                nc.gpsimd.tensor_scalar_mul(out_t[:], emin[:], m_ap)
                xb_ap = out_t[:]
            # emit sorted-order bf16 rows + original-index column; the host
            # scatters rows back to input order (tail bins are all-masked,
            # all-zero, and never emitted)
            obf = pb.tile([128, F], bf16)
            nc.vector.tensor_copy(obf[:], xb_ap)
            nc.sync.dma_start(out=out_d[s * 128:(s + 1) * 128, :], in_=obf[:])
            nc.sync.dma_start(out=oidx_d[s * 128:(s + 1) * 128, :],
                              in_=pk[:, COL_IDX:COL_IDX + 1])


def _fold_weights(inputs):
    g = inputs["ln_gamma"].astype(np.float32)
    be = inputs["ln_beta"].astype(np.float32)
    W1 = inputs["W1"].astype(np.float32)
    b1 = inputs["b1"].astype(np.float32)
    w = {
        "W1g": g[:, None] * W1,
        "b1gb": (b1 + be @ W1)[None, :],
        "W2": inputs["W2"].astype(np.float32),
        "b2": inputs["b2"].astype(np.float32)[None, :],
        "th1": inputs["th1"].astype(np.float32),
        "Wh1": inputs["Wh1"].astype(np.float32),
        "Wt1": inputs["Wt1"].astype(np.float32),
        "bt1": inputs["bt1"].astype(np.float32)[None, :],
    }
    for nm in ("th0", "Wh0", "Wt0"):
        w[nm] = g[:, None] * inputs[nm].astype(np.float32)
    w["bth0"] = (be @ inputs["th0"].astype(np.float32))[None, :]
    w["bhh0"] = (be @ inputs["Wh0"].astype(np.float32))[None, :]
    w["bgt0"] = (inputs["bt0"].astype(np.float32) +
                 be @ inputs["Wt0"].astype(np.float32))[None, :]
    return {k: np.ascontiguousarray(v, dtype=np.float32) for k, v in w.items()}


_RUNNER_CACHE = {}


def _make_runner(nc, n_cores):
    """Jit a Bass module for SPMD execution; returns the callable + metadata."""
    import jax
    from jax.sharding import Mesh, PartitionSpec, NamedSharding
    from jax.experimental.shard_map import shard_map
    from concourse import bass2jax

    partition_name = (nc.partition_id_tensor.name
                      if nc.partition_id_tensor else None)
    in_names, out_names, out_avals, zero_shapes = [], [], [], []
    for alloc in nc.m.functions[0].allocations:
        if not isinstance(alloc, mybir.MemoryLocationSet):
            continue
        name = alloc.memorylocations[0].name
        if alloc.kind == "ExternalInput":
            if name != partition_name:
                in_names.append(name)
        elif alloc.kind == "ExternalOutput":
            out_names.append(name)
            shape = tuple(alloc.tensor_shape)
            dtype = mybir.dt.np(alloc.dtype)
            out_avals.append(jax.core.ShapedArray(shape, dtype))
            zero_shapes.append((shape, dtype))
    n_params = len(in_names)
    all_names = in_names + out_names
    if partition_name is not None:
        all_names = all_names + [partition_name]

    def _body(*args):
        operands = list(args)
        if partition_name is not None:
            operands.append(bass2jax.partition_id_tensor())
        outs = bass2jax._bass_exec_p.bind(
            *operands,
            out_avals=tuple(out_avals),
            in_names=tuple(all_names),
            out_names=tuple(out_names),
            lowering_input_output_aliases=(),
            sim_require_finite=True,
            sim_require_nnan=True,
            nc=nc,
        )
        return tuple(outs)

    devices = jax.devices()[:n_cores]
    mesh = Mesh(np.asarray(devices), ("core",))
    in_specs = (PartitionSpec("core"),) * (n_params + len(out_names))
    out_specs = (PartitionSpec("core"),) * len(out_names)
    sharded = jax.jit(
        shard_map(_body, mesh=mesh, in_specs=in_specs, out_specs=out_specs,
                  check_rep=False),
        keep_unused=True)
    # zero output buffers staged on device ONCE and reused read-only
    shard = NamedSharding(mesh, PartitionSpec("core"))
    dev_zeros = [
        jax.device_put(np.zeros((n_cores * s0[0], *s0[1:]), d), shard)
        for s0, d in zero_shapes]
    return (sharded, in_names, out_names, out_avals, dev_zeros)


def _get_runners(nb, nch, ghconv_dtype, n_cores, nbu, w):
    """Cached (modK, modM) runners; weights are compile-time constants, so
    the cache key includes their fingerprint."""
    wkey = hashlib.blake2b(
        b"".join(w[k].tobytes() for k in sorted(w)), digest_size=16).hexdigest()
    key = (nb, nch, ghconv_dtype, n_cores, nbu, wkey, ABLATION)
    if key not in _RUNNER_CACHE:
        from concourse import bass2jax
        bass2jax.install_neuronx_cc_hook()
        ncK = build_keys(nb, nch, w)
        ncM = build(nb, nch, w, ghconv_dtype, nbu=nbu)
        _RUNNER_CACHE[key] = (_make_runner(ncK, n_cores),
                              _make_runner(ncM, n_cores))
    return _RUNNER_CACHE[key]


def _host_fix_keys(x2d, ridx, w, nbins):
    """Exact fp32 LSH argmax for the given row indices (matches the
    reference chain: LN (gamma/beta folded) -> ffn_dist -> argmax)."""
    xr = x2d[ridx].astype(np.float32)
    mu = xr.mean(-1, keepdims=True)
    var = ((xr - mu) ** 2).mean(-1, keepdims=True)
    zn = (xr - mu) / np.sqrt(var + 1e-6)
    h = zn @ w["W1g"] + w["b1gb"]
    h = np.where(h > 0, h, np.expm1(np.minimum(h, 0)))
    xd = h @ w["W2"] + w["b2"]
    mul = xd @ w["CB"]
    cmul = np.concatenate([mul, -mul], -1)
    return np.argmax(cmul, -1)


def run(inputs, nb, nch, n_cores, ghconv_dtype=dt.float32, trace=False,
        nbu=NBU):
    """inputs: dict with x [Btot, NP, F] float32, msk [Btot, NP] bool + weights.
    Btot must equal n_cores * nb."""
    import concurrent.futures as cf
    import jax
    from jax.sharding import Mesh, PartitionSpec, NamedSharding
    import ml_dtypes

    NP = nch * BIN
    NBINS = nch
    x = np.ascontiguousarray(inputs["x"], dtype=np.float32)
    msk = np.asarray(inputs["msk"])
    Btot = x.shape[0]
    assert Btot == n_cores * nb
    w = _fold_weights(inputs)
    w["CB"] = np.ascontiguousarray(
        inputs["codebook"][:, :NBINS // 2], dtype=np.float32)

    (rK, rM) = _get_runners(nb, nch, ghconv_dtype, n_cores, nbu, w)
    shardedK, in_namesK, out_namesK, _, dev_zerosK = rK
    shardedM, in_namesM, out_namesM, _, dev_zerosM = rM

    # ---- put: x once as bf16, shared by both dispatches ----
    x2d = x.reshape(Btot * NP, F)
    xb = x2d.astype(ml_dtypes.bfloat16)
    mf = msk.astype(np.float32).reshape(Btot * NP, 1)
    mesh = Mesh(np.asarray(jax.devices()[:n_cores]), ("core",))
    shard = NamedSharding(mesh, PartitionSpec("core"))
    xb_dev = jax.device_put(xb, shard)

    # ---- modK: LSH argmax + gap from bf16 x ----
    full = {"x": xb_dev}
    outK = shardedK(*[full[n] for n in in_namesK], *dev_zerosK)
    resK = dict(zip(out_namesK, outK))
    with cf.ThreadPoolExecutor(2) as ex:
        am_h, gap_h = ex.map(np.asarray, (resK["am"], resK["gap"]))
    am = am_h.reshape(Btot * NP).astype(np.int32)
    gap = gap_h.reshape(Btot * NP)

    # ---- host: exact argmax for risky rows, then sort ranks ----
    ridx = np.nonzero(gap < TAU)[0]
    if len(ridx):
        am[ridx] = _host_fix_keys(x2d, ridx, w, NBINS)
    keys = am.reshape(Btot, NP) + np.where(~msk, NBINS - 1, 0)
    perm = np.argsort(keys, axis=-1, kind="stable")
    ranks = np.empty((Btot, NP), np.uint32)
    ar = np.arange(NP, dtype=np.uint32)
    for bi in range(Btot):
        ranks[bi, perm[bi]] = ar

    # ---- modM: main pipeline with exact ranks ----
    full = {"x": xb_dev, "m": mf, "rank": ranks.reshape(Btot * NP, 1)}
    outM = shardedM(*[full[n] for n in in_namesM], *dev_zerosM)
    resM = dict(zip(out_namesM, outM))
    with cf.ThreadPoolExecutor(len(outM)) as ex:
        host_outs = list(ex.map(np.asarray, outM))
    resM = dict(zip(out_namesM, host_outs))

    # ---- host: scatter sorted rows back to input order ----
    out = np.zeros((Btot, NP, F), np.float32)
    for core in range(n_cores):
        for b in range(nb):
            gb = core * nb + b
            vals = resM[f"out{b}"].reshape(n_cores, nbu * BIN, F)[core]
            ids = resM[f"oidx{b}"].reshape(n_cores, nbu * BIN)[core]
            ids = ids.astype(np.int64)
            out[gb, ids] = vals.astype(np.float32)
            # every unmasked row must have been emitted within the prefix
            covered = np.zeros(NP, bool)
            covered[ids] = True
            if not (covered | ~msk[gb]).all():
                raise RuntimeError(
                    f"batch {gb}: unmasked rows beyond {nbu} sorted bins; "
                    f"increase NBU")
    return out, None


def kernel(**inputs):
    out, _ = run(inputs, nb=2, nch=100, n_cores=8)
    return out
